# revision 1
# baseline (speedup 1.0000x reference)
"""Trainium2 Bass kernel for MimiAttention (GQA + RoPE + causal softmax).

Problem: B=2, S=2048, H=1024, NH=16 q-heads, NKV=4 kv-heads, HD=64.
Sharding: 8 cores = 2 (batch) x 4 (kv-group).  Each core computes one batch's
attention for one GQA group (4 q-heads sharing 1 kv head) and the partial
o-projection for those heads; the host sums the 4 partials per batch.

Per-core device pipeline (all matmuls bf16 in / fp32 psum out):
  1. QKV projection in [d, s] layout.  RoPE is realized without any
     cross-partition shuffles by computing a second projection with
     sign-permuted weight rows (W2 rows: d<32 -> -W[d+32], d>=32 -> W[d-32]):
       q_rot = q*cos + q2*sin
     The scores contraction then uses the 128-dim identity
       q_rot . k_rot = concat(q*cos, q2*sin) . concat(k_rot, k_rot)
     so Qhat = [q*cos; q2*sin] needs only ONE elementwise multiply per chunk,
     and Khat = [k_rot; k_rot] is built by one matmul with the fold matrix
     J[p,m] = (p == m mod 64).
  2. Scores computed TRANSPOSED (scoresT[j,i]) per key-tile, exp on ACT with
     the 1/sqrt(64) scale folded in (no max subtraction needed: |s*scale|<~3),
     causal zeroing via one gpsimd affine_select per (head, j-tile).
  3. attnV: out[i, d] with lhsT = expT tile, rhs = [v | ones]: column 64 gives
     the softmax denominator as a per-partition scalar -> reciprocal + scale.
  4. o-projection after PE-transposing attn [i,c] -> [c,i]; output written
     transposed ([h, s]); host transposes back and sums partials.
"""

import numpy as np
import ml_dtypes

B, S, H = 2, 2048, 1024
NH, NKV, HD = 16, 4, 64
G = NH // NKV            # 4 q-heads per kv head
THETA = 10000.0
N_CORES = 8

BF16 = ml_dtypes.bfloat16


def _build_nc():
    import concourse.mybir as mybir
    import concourse.tile as tile
    from concourse.tile import add_dep_helper
    from concourse import bacc

    f32 = mybir.dt.float32
    bf16 = mybir.dt.bfloat16

    nc = bacc.Bacc("TRN2", target_bir_lowering=False)

    xTd = nc.dram_tensor("xT", [H, S], bf16, kind="ExternalInput")
    wqkd = nc.dram_tensor("wqkT", [H, 640], bf16, kind="ExternalInput")
    wvd = nc.dram_tensor("wvT", [H, HD], bf16, kind="ExternalInput")
    csd = nc.dram_tensor("cs", [128, S], bf16, kind="ExternalInput")
    wod = nc.dram_tensor("woT", [G * HD, H], bf16, kind="ExternalInput")
    djd = nc.dram_tensor("dupJ", [128, 128], bf16, kind="ExternalInput")
    idd = nc.dram_tensor("ident", [128, 128], bf16, kind="ExternalInput")
    trid = nc.dram_tensor("trimask", [128, 128], bf16, kind="ExternalInput")
    oTd = nc.dram_tensor("oT", [H, S], bf16, kind="ExternalOutput")

    NSB = S // 512        # 4 chunks of 512
    NST = S // 128        # 16 tiles of 128
    KC = H // 128         # 8 contraction chunks
    scale = float(1.0 / np.sqrt(HD))

    with tile.TileContext(nc) as tc:
        import contextlib
        ctx = contextlib.ExitStack()
        with ctx:
            consts = ctx.enter_context(tc.tile_pool(name="consts", bufs=1))
            acts = ctx.enter_context(tc.tile_pool(name="acts", bufs=1))
            anp = ctx.enter_context(tc.tile_pool(name="attn", bufs=1))
            rcp = ctx.enter_context(tc.tile_pool(name="rcp", bufs=6))
            etp = ctx.enter_context(tc.tile_pool(name="etri", bufs=4))
            ep = ctx.enter_context(tc.tile_pool(name="exps", bufs=1))
            otp = ctx.enter_context(tc.tile_pool(name="ot", bufs=8))
            # Shared PSUM pool: tag "s" [128,1024] fp32 x 2 slots (4 banks)
            # used by qkv waves, k-fold, v-proj, scores and the o-projection.
            psp = ctx.enter_context(
                tc.tile_pool(name="ps", bufs=4, space="PSUM"))
            pav = ctx.enter_context(
                tc.tile_pool(name="ps_av", bufs=1, space="PSUM"))
            pvp = ctx.enter_context(
                tc.tile_pool(name="ps_v", bufs=1, space="PSUM"))

            # ---- input DMAs: xt kc0 first, then weights/tables, then the
            # rest of xt (full 2048-col rows keep DMA descriptors large).
            xt_sb = consts.tile([128, KC, S], bf16, tag="xt")
            nc.sync.dma_start(xt_sb[:, 0, :], xTd[0:128, :])
            wqk_sb = consts.tile([128, KC, 640], bf16, tag="wqk")
            nc.sync.dma_start(wqk_sb, wqkd.rearrange("(kc p) m -> p kc m", p=128))
            dj_sb = consts.tile([128, 128], bf16, tag="dj")
            nc.sync.dma_start(dj_sb, djd[:, :])
            wv_sb = consts.tile([128, KC, HD], bf16, tag="wv")
            nc.sync.dma_start(wv_sb, wvd.rearrange("(kc p) m -> p kc m", p=128))
            tri_sb = consts.tile([128, 128], bf16, tag="tri")
            nc.sync.dma_start(tri_sb, trid[:, :])
            cs_sb = consts.tile([128, S], bf16, tag="cs")
            nc.sync.dma_start(cs_sb, csd[:, :])
            for kc in range(1, KC):
                nc.sync.dma_start(xt_sb[:, kc, :],
                                  xTd[kc * 128:(kc + 1) * 128, :])
            id_sb = consts.tile([128, 128], bf16, tag="id")
            nc.sync.dma_start(id_sb, idd[:, :])
            wo_sb = consts.tile([128, 2, H], bf16, tag="wo")
            nc.sync.dma_start(wo_sb, wod.rearrange("(kc p) m -> p kc m", p=128))

            qhat = [acts.tile([128, S], bf16, tag=f"qh{m}", name=f"qhat{m}")
                    for m in range(G)]
            khat = acts.tile([128, S], bf16, tag="khat")
            ktmp = acts.tile([128, S], bf16, tag="ktmp")
            v_sb = acts.tile([128, NST, HD + 1], bf16, tag="vsb")
            attn_n = [anp.tile([128, G * HD], bf16, tag=f"an{it}",
                               name=f"attn{it}")
                      for it in range(NST)]
            expT = [ep.tile([128, S], bf16, tag=f"e{jt}", name=f"expT{jt}")
                    for jt in range(NST)]
            aT = [acts.tile([128, S], bf16, tag=f"aT{c}", name=f"aTc{c}")
                  for c in range(2)]

            # attnV accumulators: slice `it` = bank[it//7][:, (it%7)*65 :+65]
            avb = [pav.tile([128, w], f32, tag=f"av{b}", name=f"avb{b}")
                   for b, w in ((0, 455), (1, 455), (2, 130))]

            def av_slice(it):
                b, o = it // 7, (it % 7) * 65
                return avb[b][:, o:o + 65]

            def proj_chunk(m, dst, n, off_slot=False):
                col = n * 512
                if off_slot:
                    ps = pvp.tile([128, 512], f32, tag="v", name="psw")
                else:
                    ps = psp.tile([128, 512], f32, tag="s", name="psw")
                for kc in range(KC):
                    nc.tensor.matmul(
                        ps, wqk_sb[:, kc, m * 128:(m + 1) * 128],
                        xt_sb[:, kc, col:col + 512],
                        start=(kc == 0), stop=(kc == KC - 1))
                nc.vector.tensor_mul(
                    dst[:, col:col + 512], ps, cs_sb[:, col:col + 512])

            def proj_wave(m, dst):
                for n in range(NSB):
                    proj_chunk(m, dst, n)

            # ---- k-side and q0 waves interleaved: both are paced by the
            # same xt DMA stream, so let them share the slot pipeline; the
            # k-fold for chunk n trails its kk2 chunk immediately.
            nc.vector.memset(v_sb[:, :, HD:HD + 1], 1.0)
            for n in range(NSB):
                proj_chunk(G, ktmp, n)
                psf = pvp.tile([128, 512], f32, tag="v", name="psf")
                nc.tensor.matmul(psf, dj_sb, ktmp[:, n * 512:(n + 1) * 512],
                                 start=True, stop=True)
                nc.vector.tensor_copy(khat[:, n * 512:(n + 1) * 512], psf)
                proj_chunk(0, qhat[0], n)

            def v_proj(st):
                psv = pvp.tile([128, HD], f32, tag="v", name="psv")
                for kc in range(KC):
                    nc.tensor.matmul(
                        psv, xt_sb[:, kc, st * 128:(st + 1) * 128],
                        wv_sb[:, kc, :],
                        start=(kc == 0), stop=(kc == KC - 1))
                nc.vector.tensor_copy(v_sb[:, st, 0:HD], psv)

            v_proj(0)

            # ---- transpose + o-projection, streamed per 512-col chunk ----
            def oproj_group(nchunk, shared=False):
                for it in range(nchunk * 4, nchunk * 4 + 4):
                    for c in range(2):
                        psx = psp.tile([128, 128], bf16, tag="s", name="pst")
                        nc.tensor.transpose(
                            psx, attn_n[it][:, c * 128:(c + 1) * 128], id_sb)
                        nc.vector.tensor_copy(
                            aT[c][:, it * 128:(it + 1) * 128], psx)
                col = nchunk * 512
                for hc in range(KC):
                    if shared or hc % 2 == 0:
                        ps2 = psp.tile([128, 512], f32, tag="s", name="pso2")
                    else:
                        ps2 = pvp.tile([128, 512], f32, tag="v", name="pso2")
                    for kc2 in range(2):
                        nc.tensor.matmul(
                            ps2, wo_sb[:, kc2, hc * 128:(hc + 1) * 128],
                            aT[kc2][:, col:col + 512],
                            start=(kc2 == 0), stop=(kc2 == 1))
                    ot = otp.tile([128, 512], bf16, tag="ot", name="otst")
                    if shared and hc % 2 == 1:
                        nc.scalar.copy(ot, ps2)
                    else:
                        nc.vector.tensor_copy(ot, ps2)
                    nc.sync.dma_start(
                        oTd[hc * 128:(hc + 1) * 128, col:col + 512], ot)

            # ---- attention: per head, per key-tile ----
            for h in range(G):
                bank_first = {}
                for jt in range(NST):
                    if h < G - 1 and jt in (1, 4, 7, 10):
                        proj_chunk(h + 1, qhat[h + 1], (jt - 1) // 3,
                                   off_slot=(h > 0))
                    lo = jt * 128
                    lhsT = khat[:, jt * 128:(jt + 1) * 128]
                    for ic in range(NSB):
                        cs_, ce = ic * 512, (ic + 1) * 512
                        if ce <= lo:
                            continue
                        s0 = max(cs_, lo)
                        ps = psp.tile([128, 512], f32, tag="s", name="pss")
                        nc.tensor.matmul(
                            ps[:, s0 - cs_:512], lhsT,
                            qhat[h][:, s0:ce], start=True, stop=True)
                        nc.scalar.activation(
                            expT[jt][:, s0:ce], ps[:, s0 - cs_:512],
                            mybir.ActivationFunctionType.Exp, scale=scale)
                    # causal triangle mask for the diagonal block (gpsimd,
                    # off the PE/ACT critical path)
                    etri = etp.tile([128, 128], bf16, tag="et", name="etri")
                    nc.gpsimd.tensor_mul(etri, expT[jt][:, lo:lo + 128],
                                         tri_sb)

                    # attnV: descending it so the masked diagonal tile is
                    # needed last; first matmul of each bank per head uses
                    # start=True (clears the bank has_written bits), all
                    # others accumulate / per-element overwrite.
                    for it in range(NST - 1, jt - 1, -1):
                        lhs = (etri if it == jt
                               else expT[jt][:, it * 128:(it + 1) * 128])
                        b = it // 7
                        first = jt == 0 and b not in bank_first
                        mm = nc.tensor.matmul(
                            av_slice(it), lhs, v_sb[:, jt, :],
                            start=first, stop=(it == jt),
                            skip_group_check=True)
                        if first:
                            bank_first[b] = mm
                        elif jt == 0:
                            add_dep_helper(mm.ins, bank_first[b].ins,
                                           sync=False,
                                           reason="bank clear first")

                    # slice it=jt is complete: normalize
                    pso = av_slice(jt)
                    rc = rcp.tile([128, 1], f32, tag="rc", name="rc")
                    nc.vector.reciprocal(rc, pso[:, HD:HD + 1])
                    nc.vector.tensor_scalar_mul(
                        attn_n[jt][:, h * HD:(h + 1) * HD], pso[:, 0:HD], rc)
                    if h == 0 and jt < NST - 1:
                        v_proj(jt + 1)
                    if h == G - 1 and jt % 4 == 3 and jt < NST - 1:
                        oproj_group(jt // 4)
            oproj_group(NSB - 1, shared=True)

    nc.finalize()
    return nc


def _host_inputs(hidden_states, position_ids, wq, wk, wv, wo):
    """Build the 8 per-core input maps."""
    def w2_of(w):
        # w: [64, H] rows of one head; returns sign-permuted rows
        w2 = np.empty_like(w)
        w2[:32] = -w[32:64]
        w2[32:] = w[:32]
        return w2

    dupJ = np.zeros((128, 128), np.float32)
    for p in range(128):
        dupJ[p, p % 64] = 1.0
        dupJ[p, p % 64 + 64] = 1.0
    dupJ = dupJ.astype(BF16)
    ident = np.eye(128, dtype=np.float32).astype(BF16)
    trimask = np.triu(np.ones((128, 128), np.float32)).astype(BF16)

    in_maps = []
    for core in range(N_CORES):
        b, kv = core // NKV, core % NKV
        xT = np.ascontiguousarray(hidden_states[b].T).astype(BF16)

        cols = []
        for i in range(G):
            h = kv * G + i
            wqh = wq[h * HD:(h + 1) * HD]
            cols.append(wqh.T)
            cols.append(w2_of(wqh).T)
        wkh = wk[kv * HD:(kv + 1) * HD]
        cols.append(wkh.T)
        cols.append(w2_of(wkh).T)
        wqkT = np.ascontiguousarray(np.concatenate(cols, axis=1)).astype(BF16)

        wvT = np.ascontiguousarray(wv[kv * HD:(kv + 1) * HD].T).astype(BF16)
        woT = np.ascontiguousarray(
            wo[:, kv * G * HD:(kv + 1) * G * HD].T).astype(BF16)

        inv = 1.0 / (THETA ** (np.arange(0, HD, 2, dtype=np.float32) / HD))
        freqs = position_ids[b].astype(np.float32)[:, None] * inv[None, :]
        emb = np.concatenate([freqs, freqs], axis=-1)       # [S, 64]
        cs = np.concatenate([np.cos(emb).T, np.sin(emb).T], axis=0)  # [128, S]
        cs = np.ascontiguousarray(cs).astype(BF16)

        in_maps.append({
            "xT": xT, "wqkT": wqkT, "wvT": wvT, "cs": cs, "woT": woT,
            "dupJ": dupJ, "ident": ident, "trimask": trimask,
        })
    return in_maps


_NC_CACHE = {}


def run_cores(in_maps, trace=False, trace_kwargs=None):
    from concourse.bass_utils import run_bass_kernel_spmd
    if "nc" not in _NC_CACHE:
        _NC_CACHE["nc"] = _build_nc()
    nc = _NC_CACHE["nc"]
    return run_bass_kernel_spmd(
        nc, in_maps, core_ids=list(range(N_CORES)),
        trace=trace, **(trace_kwargs or {}))


def kernel(hidden_states, attention_mask, position_ids, wq, wk, wv, wo):
    hidden_states = np.asarray(hidden_states, dtype=np.float32)
    position_ids = np.asarray(position_ids)
    wq = np.asarray(wq, dtype=np.float32)
    wk = np.asarray(wk, dtype=np.float32)
    wv = np.asarray(wv, dtype=np.float32)
    wo = np.asarray(wo, dtype=np.float32)

    in_maps = _host_inputs(hidden_states, position_ids, wq, wk, wv, wo)
    res = run_cores(in_maps)

    out = np.zeros((B, S, H), np.float32)
    for core in range(N_CORES):
        b = core // NKV
        out[b] += res.results[core]["oT"].T.astype(np.float32)
    return out



# revision 36
# speedup vs baseline: 1.0737x; 1.0737x over previous
"""Trainium2 Bass kernel for MimiAttention (GQA + RoPE + causal softmax).

Problem: B=2, S=2048, H=1024, NH=16 q-heads, NKV=4 kv-heads, HD=64.
Sharding: 8 cores = 2 (batch) x 4 (kv-group).  Each core computes one batch's
attention for one GQA group (4 q-heads sharing 1 kv head) and the partial
o-projection for those heads; the host sums the 4 partials per batch.

Per-core pipeline (all matmuls bf16 in / fp32 psum out):
  1. Packed QKV projection: one 128-out block holds [v; k], two blocks hold
     [q0; q1] and [q2; q3] (output width is free on the PE; only the moving
     dim costs cycles), so the whole projection is 3 blocks instead of the
     5 + separate-V of the naive RoPE-doubled layout.
  2. RoPE via PE "expand" matmuls: EP0/EP1 are constant [I | Perm] matrices
     producing [q_h; perm(q_h)] (128 rows) from a packed block; one DVE
     multiply with cs = [cos; +-sin] then yields qhat = [q cos; rot(q) sin].
     The scores contraction uses the 128-dim identity
       qhat . dup(k_rot) = q_rot . k_rot
     with khat = J @ ktmp (fold+duplicate in one matmul).
  3. Scores computed TRANSPOSED (scoresT[j,i]) into wide 2-bank PSUM tiles;
     exp on ACT in up-to-1024-wide instructions (the per-instruction PSUM /
     SBUF access overhead is ~185 ns, so wide exps matter).  Head 0 walks
     the triangle column-major (per 512 query panel) so the exp stream
     starts as soon as the first xt pieces land; heads 1-3 go row-major.
     Causal masking: one in-place Pool multiply of the diagonal tile.
  4. attnV accumulates av[i, d | denom] per 4-tile query chunk in a single
     PSUM bank ([128, 4*65] f32); reciprocal + per-partition scalar multiply
     normalizes into attn_n.
  5. attn transposes for the o-projection go through the DMA xbar
     (dma_start_transpose, no PE/PSUM involved); o-projection psum drains
     on DVE/ACT; output written transposed ([h, s]); host transposes back
     and sums partials.
"""

import numpy as np
import ml_dtypes

B, S, H = 2, 2048, 1024
NH, NKV, HD = 16, 4, 64
G = NH // NKV            # 4 q-heads per kv head
THETA = 10000.0
N_CORES = 8

BF16 = ml_dtypes.bfloat16


def _build_nc():
    import contextlib
    import concourse.mybir as mybir
    import concourse.tile as tile
    from concourse.tile import add_dep_helper
    from concourse import bacc

    f32 = mybir.dt.float32
    bf16 = mybir.dt.bfloat16
    EXP = mybir.ActivationFunctionType.Exp

    nc = bacc.Bacc("TRN2", target_bir_lowering=False)

    xTd = nc.dram_tensor("xT", [H, S], bf16, kind="ExternalInput")
    wqkvd = nc.dram_tensor("wqkvT", [H, 384], bf16, kind="ExternalInput")
    cstd = nc.dram_tensor("cst", [128, 2688], bf16, kind="ExternalInput")
    wod = nc.dram_tensor("woT", [G * HD, H], bf16, kind="ExternalInput")
    oTd = nc.dram_tensor("oT", [H, S], bf16, kind="ExternalOutput")

    KC = H // 128         # 8 contraction chunks
    NST = S // 128        # 16 tiles of 128
    scale = float(1.0 / np.sqrt(HD))

    with tile.TileContext(nc) as tc:
        ctx = contextlib.ExitStack()
        with ctx:
            consts = ctx.enter_context(tc.tile_pool(name="consts", bufs=1))
            acts = ctx.enter_context(tc.tile_pool(name="acts", bufs=1))
            rcp = ctx.enter_context(tc.tile_pool(name="rcp", bufs=6))
            otp = ctx.enter_context(tc.tile_pool(name="ot", bufs=8))
            # PSUM: sc 2x[128,1024] (4 banks) + av 2x[128,260] (2 banks)
            # + sh 2x[128,512] (2 banks) = 8 banks.
            scp = ctx.enter_context(
                tc.tile_pool(name="ps_sc", bufs=2, space="PSUM"))
            avp = ctx.enter_context(
                tc.tile_pool(name="ps_av", bufs=2, space="PSUM"))
            shp = ctx.enter_context(
                tc.tile_pool(name="ps_sh", bufs=2, space="PSUM"))

            # ---- input DMAs, ordered for earliest first matmul ----
            wqr = wqkvd.rearrange("(kc p) m -> p kc m", p=128)
            xtr = xTd.rearrange("(kc p) m -> p kc m", p=128)
            wqkv_sb = consts.tile([128, KC, 384], bf16, tag="wqkv")
            cst_sb = consts.tile([128, 2688], bf16, tag="cst")
            xt_sb = consts.tile([128, KC, S], bf16, tag="xt")
            wo_sb = consts.tile([128, 2, H], bf16, tag="wo")

            def xt_dma(i):
                nc.sync.dma_start(xt_sb[:, :, i * 512:(i + 1) * 512],
                                  xtr[:, :, i * 512:(i + 1) * 512])

            nc.sync.dma_start(cst_sb[:, 0:384], cstd[:, 0:384])
            nc.sync.dma_start(wqkv_sb[:, :, 0:256], wqr[:, :, 0:256])
            nc.sync.dma_start(cst_sb[:, 384:2688], cstd[:, 384:2688])
            for i in (0, 1, 2, 3):
                xt_dma(i)
            nc.sync.dma_start(wqkv_sb[:, :, 256:384], wqr[:, :, 256:384])
            nc.sync.dma_start(wo_sb, wod.rearrange("(kc p) m -> p kc m", p=128))

            EP0 = cst_sb[:, 0:128]
            EP1 = cst_sb[:, 128:256]
            DJ = cst_sb[:, 256:384]
            ID = cst_sb[:, 384:512]
            TRI = cst_sb[:, 512:640]
            CS = cst_sb[:, 640:2688]

            # ---- SBUF activations ----
            kv_sb = acts.tile([128, S], bf16, tag="kv")
            q01_sb = acts.tile([128, S], bf16, tag="q01")
            q23_sb = acts.tile([128, S], bf16, tag="q23")
            qhat = [acts.tile([128, S], bf16, tag=f"qh{m}", name=f"qhat{m}")
                    for m in range(G)]
            khat = acts.tile([128, S], bf16, tag="khat")
            ktmp = acts.tile([128, S], bf16, tag="ktmp")
            v_sb = acts.tile([128, NST, HD + 1], bf16, tag="vsb")
            expT = [acts.tile([128, S], bf16, tag=f"e{jt}", name=f"expT{jt}")
                    for jt in range(NST)]
            attn_n = [acts.tile([128, G * HD], bf16, tag=f"an{it}",
                                name=f"attn{it}")
                      for it in range(NST)]
            aT = [acts.tile([128, S], bf16, tag=f"aT{c}", name=f"aTc{c}")
                  for c in range(2)]
            nc.vector.memset(v_sb[:, :, HD:HD + 1], 1.0)

            cp_dve = nc.vector.tensor_copy
            cp_act = nc.scalar.copy

            # ---- helpers ----
            def proj_block(blk, n, dst, cp=None):
                ps = shp.tile([128, 512], f32, tag="sh", name="psb")
                for kc in range(KC):
                    nc.tensor.matmul(
                        ps, wqkv_sb[:, kc, blk * 128:(blk + 1) * 128],
                        xt_sb[:, kc, n * 512:(n + 1) * 512],
                        start=(kc == 0), stop=(kc == KC - 1))
                (cp or cp_dve)(dst[:, n * 512:(n + 1) * 512], ps)

            def expand(ep, src, dst, n):
                # dst chunk = (ep.T @ src chunk) * CS  -> [x; perm x] * cs
                ps = shp.tile([128, 512], f32, tag="sh", name="pse")
                nc.tensor.matmul(ps, ep, src[:, n * 512:(n + 1) * 512],
                                 start=True, stop=True)
                nc.vector.tensor_mul(dst[:, n * 512:(n + 1) * 512], ps,
                                     CS[:, n * 512:(n + 1) * 512])

            def fold(n, cp=None):
                ps = shp.tile([128, 512], f32, tag="sh", name="psf")
                nc.tensor.matmul(ps, DJ, ktmp[:, n * 512:(n + 1) * 512],
                                 start=True, stop=True)
                (cp or cp_dve)(khat[:, n * 512:(n + 1) * 512], ps)

            def v_t(st, cp=None):
                # v rows of kv block (partitions 0:64) -> v_sb[st] [128, 64]
                ps = shp.tile([128, 64], bf16, tag="sh", name="psv")
                nc.tensor.transpose(
                    ps, kv_sb[0:64, st * 128:(st + 1) * 128], ID[0:64, 0:64])
                (cp or cp_dve)(v_sb[:, st, 0:HD], ps)

            def mask_diag(jt):
                lo = jt * 128
                nc.gpsimd.tensor_mul(expT[jt][:, lo:lo + 128],
                                     expT[jt][:, lo:lo + 128], TRI)

            def scores_seg(h, jt, c0, c1):
                # query columns [c0:c1) (512-aligned), one psum tile + exp
                lo = jt * 128
                s0 = max(lo, c0)
                if s0 >= c1:
                    return
                sc = scp.tile([128, 1024], f32, tag="sc", name="scr")
                for q0_ in range(c0, c1, 512):
                    qe = q0_ + 512
                    if qe <= lo:
                        continue
                    ss = max(q0_, lo)
                    nc.tensor.matmul(
                        sc[:, ss - c0:qe - c0], khat[:, lo:lo + 128],
                        qhat[h][:, ss:qe], start=True, stop=True)
                nc.scalar.activation(expT[jt][:, s0:c1], sc[:, s0 - c0:c1 - c0],
                                     EXP, scale=scale)
                if c0 <= lo:
                    mask_diag(jt)

            # incremental attnV: per (h, chunk) a persistent psum bank;
            # contributions added per key-tile right after its exp lands
            av_state = {}

            def attnv_open(h, c):
                av = avp.tile([128, 4 * 65], f32, tag="av", name=f"av{h}_{c}")
                av_state[(h, c)] = [av, None]

            def attnv_inc(h, c, jt):
                if jt >= 4 * c + 4:
                    return
                st = av_state[(h, c)]
                av = st[0]
                for it in range(max(4 * c, jt), 4 * c + 4):
                    sl = av[:, (it - 4 * c) * 65:(it - 4 * c) * 65 + 65]
                    mm = nc.tensor.matmul(
                        sl, expT[jt][:, it * 128:(it + 1) * 128],
                        v_sb[:, jt, :], start=(st[1] is None),
                        stop=(jt == it), skip_group_check=True)
                    if st[1] is None:
                        st[1] = mm
                    else:
                        add_dep_helper(mm.ins, st[1].ins, sync=False,
                                       reason="bank clear first")

            def attnv_norm(h, c):
                av = av_state.pop((h, c))[0]
                for it in range(4 * c, 4 * c + 4):
                    sl = av[:, (it - 4 * c) * 65:(it - 4 * c) * 65 + 65]
                    rc = rcp.tile([128, 1], f32, tag="rc", name="rc")
                    nc.vector.reciprocal(rc, sl[:, HD:HD + 1])
                    nc.vector.tensor_scalar_mul(
                        attn_n[it][:, h * HD:(h + 1) * HD], sl[:, 0:HD], rc)

            def transpose_quad(c, half):
                # 4 PE transposes of chunk c's attn tiles into one psum bank,
                # drained by a single 512-wide copy
                ps = shp.tile([128, 512], bf16, tag="sh", name="pst")
                for j in range(4):
                    it = 4 * c + j
                    nc.tensor.transpose(
                        ps[:, j * 128:(j + 1) * 128],
                        attn_n[it][:, half * 128:(half + 1) * 128], ID)
                nc.vector.tensor_copy(aT[half][:, c * 512:(c + 1) * 512], ps)

            def oproj_hc(c, hc, copy_eng):
                ps = shp.tile([128, 512], f32, tag="sh", name="pso")
                for kc2 in range(2):
                    nc.tensor.matmul(
                        ps, wo_sb[:, kc2, hc * 128:(hc + 1) * 128],
                        aT[kc2][:, c * 512:(c + 1) * 512],
                        start=(kc2 == 0), stop=(kc2 == 1))
                ot = otp.tile([128, 512], bf16, tag="ot", name="otst")
                copy_eng(ot, ps)
                nc.sync.dma_start(
                    oTd[hc * 128:(hc + 1) * 128, c * 512:(c + 1) * 512], ot)

            def oproj_pair(c, hc0, copy_eng):
                # tail variant: two hc blocks through one borrowed 2-bank
                # scores tile, drained by a single wide copy
                ps = scp.tile([128, 1024], f32, tag="sc", name="psow")
                for j in range(2):
                    hc = hc0 + j
                    for kc2 in range(2):
                        nc.tensor.matmul(
                            ps[:, j * 512:(j + 1) * 512],
                            wo_sb[:, kc2, hc * 128:(hc + 1) * 128],
                            aT[kc2][:, c * 512:(c + 1) * 512],
                            start=(kc2 == 0), stop=(kc2 == 1))
                ot = otp.tile([128, 1024], bf16, tag="otw", name="otw")
                copy_eng(ot, ps)
                for j in range(2):
                    hc = hc0 + j
                    nc.sync.dma_start(
                        oTd[hc * 128:(hc + 1) * 128, c * 512:(c + 1) * 512],
                        ot[:, j * 512:(j + 1) * 512])

            cp_dve = nc.vector.tensor_copy
            cp_act = nc.scalar.copy

            # ---- lead-in: chunk 0 of kv + q01; psum->sbuf copies on the
            # scalar engine, which is idle before the exp stream ----
            proj_block(0, 0, kv_sb, cp=cp_act)
            expand(EP1, kv_sb, ktmp, 0)
            proj_block(1, 0, q01_sb, cp=cp_act)
            expand(EP0, q01_sb, qhat[0], 0)
            fold(0, cp=cp_act)
            for st in range(4):
                v_t(st)

            from collections import defaultdict
            fillA = defaultdict(list)
            fillB = defaultdict(list)

            # stripe A fillers (budget ~5.3us ACT per head window):
            # A0 sub-stripes handled inline; q1 expands late in A0
            fillA[(0, 4)].append(lambda: expand(EP1, q01_sb, qhat[1], 0))
            fillA[(0, 5)].append(lambda: expand(EP1, q01_sb, qhat[1], 1))
            fillA[(0, 6)].append(lambda: proj_block(2, 0, q23_sb))
            # A1: second q23 chunk + q2 expands (needed by A2)
            fillA[(1, 1)].append(lambda: proj_block(2, 1, q23_sb))
            fillA[(1, 4)].append(lambda: expand(EP0, q23_sb, qhat[2], 0))
            fillA[(1, 6)].append(lambda: expand(EP0, q23_sb, qhat[2], 1))
            # A2: q3 expands (needed by A3) + q01 chunk 2 + first aT quad
            fillA[(2, 0)].append(lambda: expand(EP1, q23_sb, qhat[3], 0))
            fillA[(2, 1)].append(lambda: expand(EP1, q23_sb, qhat[3], 1))
            fillA[(2, 2)].append(lambda: transpose_quad(0, 0))
            fillA[(2, 3)].append(lambda: proj_block(1, 2, q01_sb))
            fillA[(2, 6)].append(lambda: transpose_quad(1, 0))
            fillA[(2, 7)].append(lambda: expand(EP0, q01_sb, qhat[0], 2))
            # A3: q01 chunk 3 + q0/q1 expands for stripe B
            fillA[(3, 0)].append(lambda: proj_block(1, 3, q01_sb))
            fillA[(3, 4)].append(lambda: expand(EP0, q01_sb, qhat[0], 3))
            fillA[(3, 5)].append(lambda: transpose_quad(0, 1))
            fillA[(3, 6)].append(lambda: expand(EP1, q01_sb, qhat[1], 2))
            fillA[(3, 7)].append(lambda: expand(EP1, q01_sb, qhat[1], 3))

            # B0: kv chunks 2,3 chains (khat tiles 8-15, v tiles 8-15 are
            # needed from B jt8/jt12), quads + oproj group 0
            fillB[(0, 0)].append(lambda: transpose_quad(1, 1))
            fillB[(0, 1)].append(lambda: proj_block(0, 2, kv_sb))
            fillB[(0, 2)].append(lambda: expand(EP1, kv_sb, ktmp, 2))
            fillB[(0, 3)].append(lambda: fold(2))
            for st in range(8, 12):
                fillB[(0, 4)].append(lambda st=st: v_t(st))
            fillB[(0, 5)].append(lambda: proj_block(0, 3, kv_sb))
            fillB[(0, 6)].append(lambda: expand(EP1, kv_sb, ktmp, 3))
            fillB[(0, 7)].append(lambda: fold(3))
            for st in range(12, 16):
                fillB[(0, 8)].append(lambda st=st: v_t(st))
            for i in range(KC):
                fillB[(0, 9 + i // 2)].append(
                    lambda hc=i: oproj_hc(0, hc, cp_dve))
            # B1: oproj group 1 + q23 chunk 2 + q2 expand
            for i in range(KC):
                fillB[(1, 1 + i // 2)].append(
                    lambda hc=i: oproj_hc(1, hc, cp_dve))
            fillB[(1, 6)].append(lambda: proj_block(2, 2, q23_sb))
            fillB[(1, 10)].append(lambda: expand(EP0, q23_sb, qhat[2], 2))
            fillB[(1, 12)].append(lambda: proj_block(2, 3, q23_sb))
            fillB[(1, 13)].append(lambda: transpose_quad(2, 0))
            fillB[(1, 14)].append(lambda: expand(EP0, q23_sb, qhat[2], 3))
            # B2: q3 expands (needed by B3) + group-3 half0 transpose
            fillB[(2, 2)].append(lambda: transpose_quad(3, 0))
            fillB[(2, 6)].append(lambda: expand(EP1, q23_sb, qhat[3], 2))
            fillB[(2, 8)].append(lambda: expand(EP1, q23_sb, qhat[3], 3))
            # B3: group 2 transposes + first oproj hcs
            fillB[(3, 12)].append(lambda: transpose_quad(2, 1))
            for i in range(2):
                fillB[(3, 14 + i)].append(
                    lambda hc=i: oproj_hc(2, hc, cp_dve))

            # ---- stripe A: query columns [0:1024] ----
            # h0 in two sub-stripes chasing the xt DMA; h1-3 full width
            attnv_open(0, 0)
            for jt in range(4):
                scores_seg(0, jt, 0, 512)
                attnv_inc(0, 0, jt)
                if jt == 0:
                    proj_block(0, 1, kv_sb, cp=cp_act)
                if jt == 1:
                    expand(EP1, kv_sb, ktmp, 1)
                    proj_block(1, 1, q01_sb, cp=cp_act)
                if jt == 2:
                    expand(EP0, q01_sb, qhat[0], 1)
                    fold(1, cp=cp_act)
                if jt == 3:
                    attnv_norm(0, 0)
                    for st in range(4, 8):
                        v_t(st)
            attnv_open(0, 1)
            for jt in range(8):
                scores_seg(0, jt, 512, 1024)
                for f in fillA[(0, jt)]:
                    f()
                attnv_inc(0, 1, jt)
                if jt == 7:
                    attnv_norm(0, 1)
            for h in range(1, G):
                attnv_open(h, 0)
                attnv_open(h, 1)
                for jt in range(8):
                    scores_seg(h, jt, 0, 1024)
                    for f in fillA[(h, jt)]:
                        f()
                    attnv_inc(h, 0, jt)
                    attnv_inc(h, 1, jt)
                    if jt == 3:
                        attnv_norm(h, 0)
                    if jt == 7:
                        attnv_norm(h, 1)

            # ---- stripe B: query columns [1024:2048] ----
            for h in range(G):
                attnv_open(h, 2)
                attnv_open(h, 3)
                for jt in range(NST):
                    scores_seg(h, jt, 1024, 2048)
                    for f in fillB[(h, jt)]:
                        f()
                    attnv_inc(h, 2, jt)
                    attnv_inc(h, 3, jt)
                    if jt == 11:
                        attnv_norm(h, 2)
                    if jt == 15:
                        attnv_norm(h, 3)

            # ---- tail: last attn chunk's transposes + remaining oproj ----
            transpose_quad(3, 1)
            for i, hc0 in enumerate((2, 4, 6)):
                oproj_pair(2, hc0, cp_act if i % 2 == 0 else cp_dve)
            for i, hc0 in enumerate((0, 2, 4, 6)):
                oproj_pair(3, hc0, cp_act if i % 2 == 1 else cp_dve)

    nc.finalize()
    return nc


def _host_inputs(hidden_states, position_ids, wq, wk, wv, wo):
    """Build the 8 per-core input maps."""
    # expansion matrices: out[m] = src[sel(m)] with sel via one-hot columns
    EP0 = np.zeros((128, 128), np.float32)
    EP1 = np.zeros((128, 128), np.float32)
    for m in range(64):
        EP0[m, m] = 1.0
        EP0[(m + 32) % 64, 64 + m] = 1.0
        EP1[64 + m, m] = 1.0
        EP1[64 + (m + 32) % 64, 64 + m] = 1.0

    dupJ = np.zeros((128, 128), np.float32)
    for p in range(128):
        dupJ[p, p % 64] = 1.0
        dupJ[p, p % 64 + 64] = 1.0
    ident = np.eye(128, dtype=np.float32)
    trimask = np.triu(np.ones((128, 128), np.float32))

    in_maps = []
    for core in range(N_CORES):
        b, kv = core // NKV, core % NKV
        xT = np.ascontiguousarray(hidden_states[b].T).astype(BF16)

        # packed blocks: [v; k], [q0; q1], [q2; q3]
        wvh = wv[kv * HD:(kv + 1) * HD]
        wkh = wk[kv * HD:(kv + 1) * HD]
        cols = [wvh.T, wkh.T]
        for i in range(G):
            h = kv * G + i
            cols.append(wq[h * HD:(h + 1) * HD].T)
        wqkvT = np.ascontiguousarray(np.concatenate(cols, axis=1)).astype(BF16)

        woT = np.ascontiguousarray(
            wo[:, kv * G * HD:(kv + 1) * G * HD].T).astype(BF16)

        inv = 1.0 / (THETA ** (np.arange(0, HD, 2, dtype=np.float32) / HD))
        freqs = position_ids[b].astype(np.float32)[:, None] * inv[None, :]
        emb = np.concatenate([freqs, freqs], axis=-1)       # [S, 64]
        cos_t = np.cos(emb).T                               # [64, S]
        ssin_t = np.sin(emb).T.copy()
        ssin_t[:32] *= -1.0                                 # sign of rotate_half
        cs = np.concatenate([cos_t, ssin_t], axis=0)        # [128, S]

        cst = np.concatenate(
            [EP0, EP1, dupJ, ident, trimask, cs], axis=1).astype(BF16)

        in_maps.append({
            "xT": xT, "wqkvT": wqkvT, "cst": np.ascontiguousarray(cst),
            "woT": woT,
        })
    return in_maps


_NC_CACHE = {}


def run_cores(in_maps, trace=False, trace_kwargs=None):
    from concourse.bass_utils import run_bass_kernel_spmd
    if "nc" not in _NC_CACHE:
        _NC_CACHE["nc"] = _build_nc()
    nc = _NC_CACHE["nc"]
    return run_bass_kernel_spmd(
        nc, in_maps, core_ids=list(range(N_CORES)),
        trace=trace, **(trace_kwargs or {}))


def kernel(hidden_states, attention_mask, position_ids, wq, wk, wv, wo):
    hidden_states = np.asarray(hidden_states, dtype=np.float32)
    position_ids = np.asarray(position_ids)
    wq = np.asarray(wq, dtype=np.float32)
    wk = np.asarray(wk, dtype=np.float32)
    wv = np.asarray(wv, dtype=np.float32)
    wo = np.asarray(wo, dtype=np.float32)

    in_maps = _host_inputs(hidden_states, position_ids, wq, wk, wv, wo)
    res = run_cores(in_maps)

    out = np.zeros((B, S, H), np.float32)
    for core in range(N_CORES):
        b = core // NKV
        out[b] += res.results[core]["oT"].T.astype(np.float32)
    return out


# revision 42
# speedup vs baseline: 1.1311x; 1.0534x over previous
"""Trainium2 Bass kernel for MimiAttention (GQA + RoPE + causal softmax).

Problem: B=2, S=2048, H=1024, NH=16 q-heads, NKV=4 kv-heads, HD=64.
Sharding: 8 cores = 2 (batch) x 4 (kv-group).  Each core computes one batch's
attention for one GQA group (4 q-heads sharing 1 kv head) and the partial
o-projection for those heads; the host sums the 4 partials per batch.

Per-core pipeline (all matmuls bf16 in / fp32 psum out):
  1. Packed QKV projection: one 128-out block holds [v; k], two blocks hold
     [q0; q1] and [q2; q3] (output width is free on the PE; only the moving
     dim costs cycles), so the whole projection is 3 blocks instead of the
     5 + separate-V of the naive RoPE-doubled layout.
  2. RoPE via PE "expand" matmuls: EP0/EP1 are constant [I | Perm] matrices
     producing [q_h; perm(q_h)] (128 rows) from a packed block; one DVE
     multiply with cs = [cos; +-sin] then yields qhat = [q cos; rot(q) sin].
     The scores contraction uses the 128-dim identity
       qhat . dup(k_rot) = q_rot . k_rot
     with khat = J @ ktmp (fold+duplicate in one matmul).
  3. Scores computed TRANSPOSED (scoresT[j,i]) into wide 2-bank PSUM tiles;
     exp on ACT in up-to-1024-wide instructions (the per-instruction PSUM /
     SBUF access overhead is ~185 ns, so wide exps matter).  Head 0 walks
     the triangle column-major (per 512 query panel) so the exp stream
     starts as soon as the first xt pieces land; heads 1-3 go row-major.
     Causal masking: one in-place Pool multiply of the diagonal tile.
  4. attnV accumulates av[i, d | denom] per 4-tile query chunk in a single
     PSUM bank ([128, 4*65] f32); reciprocal + per-partition scalar multiply
     normalizes into attn_n.
  5. attn transposes for the o-projection go through the DMA xbar
     (dma_start_transpose, no PE/PSUM involved); o-projection psum drains
     on DVE/ACT; output written transposed ([h, s]); host transposes back
     and sums partials.
"""

import numpy as np
import ml_dtypes

B, S, H = 2, 2048, 1024
NH, NKV, HD = 16, 4, 64
G = NH // NKV            # 4 q-heads per kv head
THETA = 10000.0
N_CORES = 8

BF16 = ml_dtypes.bfloat16


def _build_nc():
    import contextlib
    import concourse.mybir as mybir
    import concourse.tile as tile
    from concourse.tile import add_dep_helper
    from concourse import bacc

    f32 = mybir.dt.float32
    bf16 = mybir.dt.bfloat16
    EXP = mybir.ActivationFunctionType.Exp

    nc = bacc.Bacc("TRN2", target_bir_lowering=False)

    xTd = nc.dram_tensor("xT", [H, S], bf16, kind="ExternalInput")
    wqkvd = nc.dram_tensor("wqkvT", [H, 384], bf16, kind="ExternalInput")
    cstd = nc.dram_tensor("cst", [128, 2688], bf16, kind="ExternalInput")
    wod = nc.dram_tensor("woT", [G * HD, H], bf16, kind="ExternalInput")
    oTd = nc.dram_tensor("oT", [H, S], bf16, kind="ExternalOutput")

    KC = H // 128         # 8 contraction chunks
    NST = S // 128        # 16 tiles of 128
    scale = float(1.0 / np.sqrt(HD))

    with tile.TileContext(nc) as tc:
        ctx = contextlib.ExitStack()
        with ctx:
            consts = ctx.enter_context(tc.tile_pool(name="consts", bufs=1))
            acts = ctx.enter_context(tc.tile_pool(name="acts", bufs=1))
            rcp = ctx.enter_context(tc.tile_pool(name="rcp", bufs=6))
            otp = ctx.enter_context(tc.tile_pool(name="ot", bufs=8))
            # PSUM: sc 2x[128,1024] (4 banks) + av 2x[128,260] (2 banks)
            # + sh 2x[128,512] (2 banks) = 8 banks.
            scp = ctx.enter_context(
                tc.tile_pool(name="ps_sc", bufs=2, space="PSUM"))
            avp = ctx.enter_context(
                tc.tile_pool(name="ps_av", bufs=2, space="PSUM"))
            shp = ctx.enter_context(
                tc.tile_pool(name="ps_sh", bufs=2, space="PSUM"))

            # ---- input DMAs, ordered for earliest first matmul ----
            wqr = wqkvd.rearrange("(kc p) m -> p kc m", p=128)
            xtr = xTd.rearrange("(kc p) m -> p kc m", p=128)
            wqkv_sb = consts.tile([128, KC, 384], bf16, tag="wqkv")
            cst_sb = consts.tile([128, 2688], bf16, tag="cst")
            xt_sb = consts.tile([128, KC, S], bf16, tag="xt")
            wo_sb = consts.tile([128, 2, H], bf16, tag="wo")

            def xt_dma(i):
                nc.sync.dma_start(xt_sb[:, :, i * 512:(i + 1) * 512],
                                  xtr[:, :, i * 512:(i + 1) * 512])

            nc.sync.dma_start(cst_sb[:, 0:384], cstd[:, 0:384])
            nc.sync.dma_start(wqkv_sb[:, :, 0:256], wqr[:, :, 0:256])
            nc.sync.dma_start(cst_sb[:, 384:2688], cstd[:, 384:2688])
            for i in (0, 1, 2, 3):
                xt_dma(i)
            nc.sync.dma_start(wqkv_sb[:, :, 256:384], wqr[:, :, 256:384])
            nc.sync.dma_start(wo_sb, wod.rearrange("(kc p) m -> p kc m", p=128))

            EP0 = cst_sb[:, 0:128]
            EP1 = cst_sb[:, 128:256]
            DJ = cst_sb[:, 256:384]
            ID = cst_sb[:, 384:512]
            TRI = cst_sb[:, 512:640]
            CS = cst_sb[:, 640:2688]

            # ---- SBUF activations ----
            kv_sb = acts.tile([128, S], bf16, tag="kv")
            q01_sb = acts.tile([128, S], bf16, tag="q01")
            q23_sb = acts.tile([128, S], bf16, tag="q23")
            qhat = [acts.tile([128, S], bf16, tag=f"qh{m}", name=f"qhat{m}")
                    for m in range(G)]
            khat = acts.tile([128, S], bf16, tag="khat")
            ktmp = acts.tile([128, S], bf16, tag="ktmp")
            v_sb = acts.tile([128, NST, HD + 1], bf16, tag="vsb")
            expT = [acts.tile([128, S], bf16, tag=f"e{jt}", name=f"expT{jt}")
                    for jt in range(NST)]
            attn_n = [acts.tile([128, G * HD], bf16, tag=f"an{it}",
                                name=f"attn{it}")
                      for it in range(NST)]
            aT = [acts.tile([128, S], bf16, tag=f"aT{c}", name=f"aTc{c}")
                  for c in range(2)]
            wup = acts.tile([128, 512], bf16, tag="wup")
            nc.vector.memset(wup, 0.0)
            nc.vector.memset(v_sb[:, :, HD:HD + 1], 1.0)

            cp_dve = nc.vector.tensor_copy
            cp_act = nc.scalar.copy

            # PE p-state warmup: junk matmuls bridge the DMA wait so the
            # tensor engine is at full clock when real projections start.
            for _ in range(16):
                pw = scp.tile([128, 1024], f32, tag="sc", name="pwarm")
                nc.tensor.matmul(pw[:, 0:512], wup[:, 0:128], wup,
                                 start=True, stop=True)

            # ---- helpers ----
            _proj_ps = {}

            def proj_block(blk, n, dst, cp=None, part=2):
                # part 0: first half of the contraction; 1: second half +
                # drain; 2: whole block.  Halves let an 8-matmul projection
                # spread over two filler slots.
                if part in (0, 2):
                    _proj_ps[(blk, n)] = shp.tile(
                        [128, 512], f32, tag="sh", name="psb")
                ps = _proj_ps[(blk, n)]
                kcs = {0: range(0, 4), 1: range(4, KC), 2: range(KC)}[part]
                for kc in kcs:
                    nc.tensor.matmul(
                        ps, wqkv_sb[:, kc, blk * 128:(blk + 1) * 128],
                        xt_sb[:, kc, n * 512:(n + 1) * 512],
                        start=(kc == 0), stop=(kc == KC - 1))
                if part in (1, 2):
                    (cp or cp_dve)(dst[:, n * 512:(n + 1) * 512], ps)
                    del _proj_ps[(blk, n)]

            def expand(ep, src, dst, n):
                # dst chunk = (ep.T @ src chunk) * CS  -> [x; perm x] * cs
                ps = shp.tile([128, 512], f32, tag="sh", name="pse")
                nc.tensor.matmul(ps, ep, src[:, n * 512:(n + 1) * 512],
                                 start=True, stop=True)
                nc.vector.tensor_mul(dst[:, n * 512:(n + 1) * 512], ps,
                                     CS[:, n * 512:(n + 1) * 512])

            def fold(n, cp=None):
                ps = shp.tile([128, 512], f32, tag="sh", name="psf")
                nc.tensor.matmul(ps, DJ, ktmp[:, n * 512:(n + 1) * 512],
                                 start=True, stop=True)
                (cp or cp_dve)(khat[:, n * 512:(n + 1) * 512], ps)

            def v_t(st, cp=None):
                # v rows of kv block (partitions 0:64) -> v_sb[st] [128, 64]
                ps = shp.tile([128, 64], bf16, tag="sh", name="psv")
                nc.tensor.transpose(
                    ps, kv_sb[0:64, st * 128:(st + 1) * 128], ID[0:64, 0:64])
                (cp or cp_dve)(v_sb[:, st, 0:HD], ps)

            def mask_diag(jt):
                lo = jt * 128
                nc.gpsimd.tensor_mul(expT[jt][:, lo:lo + 128],
                                     expT[jt][:, lo:lo + 128], TRI)

            def scores_seg(h, jt, c0, c1):
                # query columns [c0:c1) (512-aligned), one psum tile + exp
                lo = jt * 128
                s0 = max(lo, c0)
                if s0 >= c1:
                    return
                sc = scp.tile([128, 1024], f32, tag="sc", name="scr")
                for q0_ in range(c0, c1, 512):
                    qe = q0_ + 512
                    if qe <= lo:
                        continue
                    ss = max(q0_, lo)
                    nc.tensor.matmul(
                        sc[:, ss - c0:qe - c0], khat[:, lo:lo + 128],
                        qhat[h][:, ss:qe], start=True, stop=True)
                nc.scalar.activation(expT[jt][:, s0:c1], sc[:, s0 - c0:c1 - c0],
                                     EXP, scale=scale)
                if c0 <= lo:
                    mask_diag(jt)

            # incremental attnV: per (h, chunk) a persistent psum bank;
            # contributions added per key-tile right after its exp lands
            av_state = {}

            def attnv_open(h, c):
                av = avp.tile([128, 4 * 65], f32, tag="av", name=f"av{h}_{c}")
                av_state[(h, c)] = [av, None]

            def attnv_inc(h, c, jt):
                if jt >= 4 * c + 4:
                    return
                st = av_state[(h, c)]
                av = st[0]
                for it in range(max(4 * c, jt), 4 * c + 4):
                    sl = av[:, (it - 4 * c) * 65:(it - 4 * c) * 65 + 65]
                    mm = nc.tensor.matmul(
                        sl, expT[jt][:, it * 128:(it + 1) * 128],
                        v_sb[:, jt, :], start=(st[1] is None),
                        stop=(jt == it), skip_group_check=True)
                    if st[1] is None:
                        st[1] = mm
                    else:
                        add_dep_helper(mm.ins, st[1].ins, sync=False,
                                       reason="bank clear first")

            def attnv_norm(h, c):
                av = av_state.pop((h, c))[0]
                for it in range(4 * c, 4 * c + 4):
                    sl = av[:, (it - 4 * c) * 65:(it - 4 * c) * 65 + 65]
                    rc = rcp.tile([128, 1], f32, tag="rc", name="rc")
                    nc.vector.reciprocal(rc, sl[:, HD:HD + 1])
                    nc.vector.tensor_scalar_mul(
                        attn_n[it][:, h * HD:(h + 1) * HD], sl[:, 0:HD], rc)

            def transpose_quad(c, half):
                # 4 PE transposes of chunk c's attn tiles into one psum bank,
                # drained by a single 512-wide copy
                ps = shp.tile([128, 512], bf16, tag="sh", name="pst")
                for j in range(4):
                    it = 4 * c + j
                    nc.tensor.transpose(
                        ps[:, j * 128:(j + 1) * 128],
                        attn_n[it][:, half * 128:(half + 1) * 128], ID)
                nc.vector.tensor_copy(aT[half][:, c * 512:(c + 1) * 512], ps)

            def oproj_hc(c, hc, copy_eng):
                ps = shp.tile([128, 512], f32, tag="sh", name="pso")
                for kc2 in range(2):
                    nc.tensor.matmul(
                        ps, wo_sb[:, kc2, hc * 128:(hc + 1) * 128],
                        aT[kc2][:, c * 512:(c + 1) * 512],
                        start=(kc2 == 0), stop=(kc2 == 1))
                ot = otp.tile([128, 512], bf16, tag="ot", name="otst")
                copy_eng(ot, ps)
                nc.sync.dma_start(
                    oTd[hc * 128:(hc + 1) * 128, c * 512:(c + 1) * 512], ot)

            def oproj_pair(c, hc0, copy_eng):
                # tail variant: two hc blocks through one borrowed 2-bank
                # scores tile, drained by a single wide copy
                ps = scp.tile([128, 1024], f32, tag="sc", name="psow")
                for j in range(2):
                    hc = hc0 + j
                    for kc2 in range(2):
                        nc.tensor.matmul(
                            ps[:, j * 512:(j + 1) * 512],
                            wo_sb[:, kc2, hc * 128:(hc + 1) * 128],
                            aT[kc2][:, c * 512:(c + 1) * 512],
                            start=(kc2 == 0), stop=(kc2 == 1))
                ot = otp.tile([128, 1024], bf16, tag="otw", name="otw")
                copy_eng(ot, ps)
                for j in range(2):
                    hc = hc0 + j
                    nc.sync.dma_start(
                        oTd[hc * 128:(hc + 1) * 128, c * 512:(c + 1) * 512],
                        ot[:, j * 512:(j + 1) * 512])

            cp_dve = nc.vector.tensor_copy
            cp_act = nc.scalar.copy

            # ---- lead-in: chunks 0,1 of kv + q01; psum->sbuf copies on the
            # scalar engine, which is idle before the exp stream ----
            proj_block(0, 0, kv_sb, cp=cp_act)
            expand(EP1, kv_sb, ktmp, 0)
            proj_block(1, 0, q01_sb, cp=cp_act)
            expand(EP0, q01_sb, qhat[0], 0)
            fold(0, cp=cp_act)
            proj_block(0, 1, kv_sb, cp=cp_act)
            expand(EP1, kv_sb, ktmp, 1)
            proj_block(1, 1, q01_sb, cp=cp_act)
            expand(EP0, q01_sb, qhat[0], 1)
            fold(1, cp=cp_act)
            for st in range(4):
                v_t(st, cp=cp_act)

            from collections import defaultdict
            fillA = defaultdict(list)
            fillB = defaultdict(list)

            # stripe A fillers (budget ~5.3us ACT per head window); chain-
            # dependent ops spaced >=2 jts apart, 8-matmul projections split
            # into halves (adjacent slots, no intervening shp allocation)
            fillA[(0, 1)].append(lambda: expand(EP1, q01_sb, qhat[1], 0))
            fillA[(0, 2)].append(lambda: expand(EP1, q01_sb, qhat[1], 1))
            for st in range(4, 8):
                fillA[(0, 3)].append(lambda st=st: v_t(st))
            fillA[(0, 4)].append(lambda: proj_block(2, 0, q23_sb, part=0))
            fillA[(0, 5)].append(lambda: proj_block(2, 0, q23_sb, part=1))
            fillA[(0, 6)].append(lambda: proj_block(2, 1, q23_sb, part=0))
            fillA[(0, 7)].append(lambda: proj_block(2, 1, q23_sb, part=1))
            # A1: q2/q3 expands (needed by A2/A3)
            fillA[(1, 1)].append(lambda: expand(EP0, q23_sb, qhat[2], 0))
            fillA[(1, 3)].append(lambda: expand(EP0, q23_sb, qhat[2], 1))
            fillA[(1, 5)].append(lambda: expand(EP1, q23_sb, qhat[3], 0))
            fillA[(1, 7)].append(lambda: expand(EP1, q23_sb, qhat[3], 1))
            # A2: q01 chunks 2,3 + first aT quads
            fillA[(2, 0)].append(lambda: proj_block(1, 2, q01_sb, part=0))
            fillA[(2, 1)].append(lambda: proj_block(1, 2, q01_sb, part=1))
            fillA[(2, 2)].append(lambda: transpose_quad(0, 0))
            fillA[(2, 3)].append(lambda: proj_block(1, 3, q01_sb, part=0))
            fillA[(2, 4)].append(lambda: proj_block(1, 3, q01_sb, part=1))
            fillA[(2, 5)].append(lambda: expand(EP0, q01_sb, qhat[0], 2))
            fillA[(2, 6)].append(lambda: transpose_quad(1, 0))
            fillA[(2, 7)].append(lambda: expand(EP1, q01_sb, qhat[1], 2))
            # A3: q0/q1 expands for stripe B
            fillA[(3, 0)].append(lambda: expand(EP0, q01_sb, qhat[0], 3))
            fillA[(3, 2)].append(lambda: expand(EP1, q01_sb, qhat[1], 3))
            fillA[(3, 5)].append(lambda: transpose_quad(0, 1))

            # B0: kv chunks 2,3 chains (khat tiles 8-15, v tiles 8-15 are
            # needed from B jt8/jt12) + last chunk-1 quad
            fillB[(0, 0)].append(lambda: transpose_quad(1, 1))
            fillB[(0, 0)].append(lambda: proj_block(0, 2, kv_sb, part=0))
            fillB[(0, 1)].append(lambda: proj_block(0, 2, kv_sb, part=1))
            fillB[(0, 2)].append(lambda: expand(EP1, kv_sb, ktmp, 2))
            fillB[(0, 4)].append(lambda: fold(2))
            for st in range(8, 10):
                fillB[(0, 4)].append(lambda st=st: v_t(st))
            for st in range(10, 12):
                fillB[(0, 5)].append(lambda st=st: v_t(st))
            fillB[(0, 5)].append(lambda: proj_block(0, 3, kv_sb, part=0))
            fillB[(0, 6)].append(lambda: proj_block(0, 3, kv_sb, part=1))
            fillB[(0, 7)].append(lambda: expand(EP1, kv_sb, ktmp, 3))
            fillB[(0, 9)].append(lambda: fold(3))
            for st in range(12, 16):
                fillB[(0, 10)].append(lambda st=st: v_t(st))
            # B1: oproj group 0 (1 hc/jt) + q23 chunks 2,3
            for i in range(KC):
                fillB[(1, i)].append(
                    lambda hc=i: oproj_hc(0, hc, cp_dve))
            fillB[(1, 9)].append(lambda: proj_block(2, 2, q23_sb))
            fillB[(1, 11)].append(lambda: expand(EP0, q23_sb, qhat[2], 2))
            fillB[(1, 13)].append(lambda: proj_block(2, 3, q23_sb))
            fillB[(1, 14)].append(lambda: transpose_quad(2, 0))
            fillB[(1, 15)].append(lambda: expand(EP0, q23_sb, qhat[2], 3))
            # B2: oproj group 1 + q3 expands (needed by B3)
            for i in range(KC):
                fillB[(2, i)].append(
                    lambda hc=i: oproj_hc(1, hc, cp_dve))
            fillB[(2, 9)].append(lambda: transpose_quad(3, 0))
            fillB[(2, 11)].append(lambda: expand(EP1, q23_sb, qhat[3], 2))
            fillB[(2, 13)].append(lambda: expand(EP1, q23_sb, qhat[3], 3))
            # B3: group 2 transposes + first oproj hcs
            fillB[(3, 12)].append(lambda: transpose_quad(2, 1))
            for i in range(2):
                fillB[(3, 14 + i)].append(
                    lambda hc=i: oproj_hc(2, hc, cp_dve))

            # ---- stripe A: query columns [0:1024] ----
            for h in range(G):
                attnv_open(h, 0)
                attnv_open(h, 1)
                for jt in range(8):
                    scores_seg(h, jt, 0, 1024)
                    for f in fillA[(h, jt)]:
                        f()
                    attnv_inc(h, 0, jt)
                    attnv_inc(h, 1, jt)
                    if jt == 3:
                        attnv_norm(h, 0)
                    if jt == 7:
                        attnv_norm(h, 1)

            # ---- stripe B: query columns [1024:2048] ----
            for h in range(G):
                attnv_open(h, 2)
                attnv_open(h, 3)
                for jt in range(NST):
                    scores_seg(h, jt, 1024, 2048)
                    for f in fillB[(h, jt)]:
                        f()
                    attnv_inc(h, 2, jt)
                    attnv_inc(h, 3, jt)
                    if jt == 11:
                        attnv_norm(h, 2)
                    if jt == 15:
                        attnv_norm(h, 3)

            # ---- tail: last attn chunk's transposes + remaining oproj ----
            transpose_quad(3, 1)
            for i, hc0 in enumerate((2, 4, 6)):
                oproj_pair(2, hc0, cp_act if i % 2 == 0 else cp_dve)
            for i, hc0 in enumerate((0, 2, 4, 6)):
                oproj_pair(3, hc0, cp_act if i % 2 == 1 else cp_dve)

    nc.finalize()
    return nc


def _host_inputs(hidden_states, position_ids, wq, wk, wv, wo):
    """Build the 8 per-core input maps."""
    # expansion matrices: out[m] = src[sel(m)] with sel via one-hot columns
    EP0 = np.zeros((128, 128), np.float32)
    EP1 = np.zeros((128, 128), np.float32)
    for m in range(64):
        EP0[m, m] = 1.0
        EP0[(m + 32) % 64, 64 + m] = 1.0
        EP1[64 + m, m] = 1.0
        EP1[64 + (m + 32) % 64, 64 + m] = 1.0

    dupJ = np.zeros((128, 128), np.float32)
    for p in range(128):
        dupJ[p, p % 64] = 1.0
        dupJ[p, p % 64 + 64] = 1.0
    ident = np.eye(128, dtype=np.float32)
    trimask = np.triu(np.ones((128, 128), np.float32))

    in_maps = []
    for core in range(N_CORES):
        b, kv = core // NKV, core % NKV
        xT = np.ascontiguousarray(hidden_states[b].T).astype(BF16)

        # packed blocks: [v; k], [q0; q1], [q2; q3]
        wvh = wv[kv * HD:(kv + 1) * HD]
        wkh = wk[kv * HD:(kv + 1) * HD]
        cols = [wvh.T, wkh.T]
        for i in range(G):
            h = kv * G + i
            cols.append(wq[h * HD:(h + 1) * HD].T)
        wqkvT = np.ascontiguousarray(np.concatenate(cols, axis=1)).astype(BF16)

        woT = np.ascontiguousarray(
            wo[:, kv * G * HD:(kv + 1) * G * HD].T).astype(BF16)

        inv = 1.0 / (THETA ** (np.arange(0, HD, 2, dtype=np.float32) / HD))
        freqs = position_ids[b].astype(np.float32)[:, None] * inv[None, :]
        emb = np.concatenate([freqs, freqs], axis=-1)       # [S, 64]
        cos_t = np.cos(emb).T                               # [64, S]
        ssin_t = np.sin(emb).T.copy()
        ssin_t[:32] *= -1.0                                 # sign of rotate_half
        cs = np.concatenate([cos_t, ssin_t], axis=0)        # [128, S]

        cst = np.concatenate(
            [EP0, EP1, dupJ, ident, trimask, cs], axis=1).astype(BF16)

        in_maps.append({
            "xT": xT, "wqkvT": wqkvT, "cst": np.ascontiguousarray(cst),
            "woT": woT,
        })
    return in_maps


_NC_CACHE = {}


def run_cores(in_maps, trace=False, trace_kwargs=None):
    from concourse.bass_utils import run_bass_kernel_spmd
    if "nc" not in _NC_CACHE:
        _NC_CACHE["nc"] = _build_nc()
    nc = _NC_CACHE["nc"]
    return run_bass_kernel_spmd(
        nc, in_maps, core_ids=list(range(N_CORES)),
        trace=trace, **(trace_kwargs or {}))


def kernel(hidden_states, attention_mask, position_ids, wq, wk, wv, wo):
    hidden_states = np.asarray(hidden_states, dtype=np.float32)
    position_ids = np.asarray(position_ids)
    wq = np.asarray(wq, dtype=np.float32)
    wk = np.asarray(wk, dtype=np.float32)
    wv = np.asarray(wv, dtype=np.float32)
    wo = np.asarray(wo, dtype=np.float32)

    in_maps = _host_inputs(hidden_states, position_ids, wq, wk, wv, wo)
    res = run_cores(in_maps)

    out = np.zeros((B, S, H), np.float32)
    for core in range(N_CORES):
        b = core // NKV
        out[b] += res.results[core]["oT"].T.astype(np.float32)
    return out


# revision 48
# speedup vs baseline: 1.1440x; 1.0115x over previous
"""Trainium2 Bass kernel for MimiAttention (GQA + RoPE + causal softmax).

Problem: B=2, S=2048, H=1024, NH=16 q-heads, NKV=4 kv-heads, HD=64.
Sharding: 8 cores = 2 (batch) x 4 (kv-group).  Each core computes one batch's
attention for one GQA group (4 q-heads sharing 1 kv head) and the partial
o-projection for those heads; the host sums the 4 partials per batch.

Per-core pipeline (all matmuls bf16 in / fp32 psum out):
  1. Packed QKV projection: one 128-out block holds [v; k], two blocks hold
     [q0; q1] and [q2; q3] (output width is free on the PE; only the moving
     dim costs cycles), so the whole projection is 3 blocks instead of the
     5 + separate-V of the naive RoPE-doubled layout.
  2. RoPE via PE "expand" matmuls: EP0/EP1 are constant [I | Perm] matrices
     producing [q_h; perm(q_h)] (128 rows) from a packed block; one DVE
     multiply with cs = [cos; +-sin] then yields qhat = [q cos; rot(q) sin].
     The scores contraction uses the 128-dim identity
       qhat . dup(k_rot) = q_rot . k_rot
     with khat = J @ ktmp (fold+duplicate in one matmul).
  3. Scores computed TRANSPOSED (scoresT[j,i]) into wide 2-bank PSUM tiles;
     exp on ACT in up-to-1024-wide instructions (the per-instruction PSUM /
     SBUF access overhead is ~185 ns, so wide exps matter).  Head 0 walks
     the triangle column-major (per 512 query panel) so the exp stream
     starts as soon as the first xt pieces land; heads 1-3 go row-major.
     Causal masking: one in-place Pool multiply of the diagonal tile.
  4. attnV accumulates av[i, d | denom] per 4-tile query chunk in a single
     PSUM bank ([128, 4*65] f32); reciprocal + per-partition scalar multiply
     normalizes into attn_n.
  5. attn transposes for the o-projection go through the DMA xbar
     (dma_start_transpose, no PE/PSUM involved); o-projection psum drains
     on DVE/ACT; output written transposed ([h, s]); host transposes back
     and sums partials.
"""

import numpy as np
import ml_dtypes

B, S, H = 2, 2048, 1024
NH, NKV, HD = 16, 4, 64
G = NH // NKV            # 4 q-heads per kv head
THETA = 10000.0
N_CORES = 8

BF16 = ml_dtypes.bfloat16


def _build_nc():
    import contextlib
    import concourse.mybir as mybir
    import concourse.tile as tile
    from concourse.tile import add_dep_helper
    from concourse import bacc

    f32 = mybir.dt.float32
    bf16 = mybir.dt.bfloat16
    EXP = mybir.ActivationFunctionType.Exp

    nc = bacc.Bacc("TRN2", target_bir_lowering=False)

    xTd = nc.dram_tensor("xT", [H, S], bf16, kind="ExternalInput")
    wqkvd = nc.dram_tensor("wqkvT", [H, 384], bf16, kind="ExternalInput")
    cstd = nc.dram_tensor("cst", [128, 2688], bf16, kind="ExternalInput")
    wod = nc.dram_tensor("woT", [G * HD, H], bf16, kind="ExternalInput")
    oTd = nc.dram_tensor("oT", [H, S], bf16, kind="ExternalOutput")

    KC = H // 128         # 8 contraction chunks
    NST = S // 128        # 16 tiles of 128
    scale = float(1.0 / np.sqrt(HD))

    with tile.TileContext(nc) as tc:
        ctx = contextlib.ExitStack()
        with ctx:
            consts = ctx.enter_context(tc.tile_pool(name="consts", bufs=1))
            acts = ctx.enter_context(tc.tile_pool(name="acts", bufs=1))
            rcp = ctx.enter_context(tc.tile_pool(name="rcp", bufs=6))
            otp = ctx.enter_context(tc.tile_pool(name="ot", bufs=8))
            # PSUM: sc 2x[128,1024] (4 banks) + av 2x[128,260] (2 banks)
            # + sh 2x[128,512] (2 banks) = 8 banks.
            scp = ctx.enter_context(
                tc.tile_pool(name="ps_sc", bufs=2, space="PSUM"))
            avp = ctx.enter_context(
                tc.tile_pool(name="ps_av", bufs=2, space="PSUM"))
            shp = ctx.enter_context(
                tc.tile_pool(name="ps_sh", bufs=2, space="PSUM"))

            # ---- input DMAs, ordered for earliest first matmul ----
            wqr = wqkvd.rearrange("(kc p) m -> p kc m", p=128)
            xtr = xTd.rearrange("(kc p) m -> p kc m", p=128)
            wqkv_sb = consts.tile([128, KC, 384], bf16, tag="wqkv")
            cst_sb = consts.tile([128, 2688], bf16, tag="cst")
            xt_sb = consts.tile([128, KC, S], bf16, tag="xt")
            wo_sb = consts.tile([128, 2, H], bf16, tag="wo")

            def xt_dma(i):
                nc.sync.dma_start(xt_sb[:, :, i * 512:(i + 1) * 512],
                                  xtr[:, :, i * 512:(i + 1) * 512])

            nc.sync.dma_start(cst_sb[:, 0:384], cstd[:, 0:384])
            nc.sync.dma_start(wqkv_sb[:, :, 0:256], wqr[:, :, 0:256])
            xt_dma(0)
            xt_dma(1)
            nc.sync.dma_start(cst_sb[:, 384:2688], cstd[:, 384:2688])
            xt_dma(2)
            xt_dma(3)
            nc.sync.dma_start(wqkv_sb[:, :, 256:384], wqr[:, :, 256:384])
            nc.sync.dma_start(wo_sb, wod.rearrange("(kc p) m -> p kc m", p=128))

            EP0 = cst_sb[:, 0:128]
            EP1 = cst_sb[:, 128:256]
            DJ = cst_sb[:, 256:384]
            ID = cst_sb[:, 384:512]
            TRI = cst_sb[:, 512:640]
            CS = cst_sb[:, 640:2688]

            # ---- SBUF activations ----
            kv_sb = acts.tile([128, S], bf16, tag="kv")
            q01_sb = acts.tile([128, S], bf16, tag="q01")
            q23_sb = acts.tile([128, S], bf16, tag="q23")
            qhat = [acts.tile([128, S], bf16, tag=f"qh{m}", name=f"qhat{m}")
                    for m in range(G)]
            khat = acts.tile([128, S], bf16, tag="khat")
            ktmp = acts.tile([128, S], bf16, tag="ktmp")
            v_sb = acts.tile([128, NST, HD + 1], bf16, tag="vsb")
            expT = [acts.tile([128, S], bf16, tag=f"e{jt}", name=f"expT{jt}")
                    for jt in range(NST)]
            attn_n = [acts.tile([128, G * HD], bf16, tag=f"an{it}",
                                name=f"attn{it}")
                      for it in range(NST)]
            aT = [acts.tile([128, S], bf16, tag=f"aT{c}", name=f"aTc{c}")
                  for c in range(2)]
            wup = acts.tile([128, 512], bf16, tag="wup")
            nc.vector.memset(wup, 0.0)
            nc.vector.memset(v_sb[:, :, HD:HD + 1], 1.0)

            cp_dve = nc.vector.tensor_copy
            cp_act = nc.scalar.copy

            # PE p-state warmup: junk matmuls bridge the DMA wait so the
            # tensor engine is at full clock when real projections start.
            for _ in range(16):
                pw = scp.tile([128, 1024], f32, tag="sc", name="pwarm")
                nc.tensor.matmul(pw[:, 0:512], wup[:, 0:128], wup,
                                 start=True, stop=True)

            # ---- helpers ----
            _proj_ps = {}

            def proj_block(blk, n, dst, cp=None, part=2):
                # part 0: first half of the contraction; 1: second half +
                # drain; 2: whole block.  Halves let an 8-matmul projection
                # spread over two filler slots.
                if part in (0, 2):
                    _proj_ps[(blk, n)] = shp.tile(
                        [128, 512], f32, tag="sh", name="psb")
                ps = _proj_ps[(blk, n)]
                kcs = {0: range(0, 4), 1: range(4, KC), 2: range(KC)}[part]
                for kc in kcs:
                    nc.tensor.matmul(
                        ps, wqkv_sb[:, kc, blk * 128:(blk + 1) * 128],
                        xt_sb[:, kc, n * 512:(n + 1) * 512],
                        start=(kc == 0), stop=(kc == KC - 1))
                if part in (1, 2):
                    (cp or cp_dve)(dst[:, n * 512:(n + 1) * 512], ps)
                    del _proj_ps[(blk, n)]

            def expand(ep, src, dst, n):
                # dst chunk = (ep.T @ src chunk) * CS  -> [x; perm x] * cs
                ps = shp.tile([128, 512], f32, tag="sh", name="pse")
                nc.tensor.matmul(ps, ep, src[:, n * 512:(n + 1) * 512],
                                 start=True, stop=True)
                nc.vector.tensor_mul(dst[:, n * 512:(n + 1) * 512], ps,
                                     CS[:, n * 512:(n + 1) * 512])

            def fold(n, cp=None):
                ps = shp.tile([128, 512], f32, tag="sh", name="psf")
                nc.tensor.matmul(ps, DJ, ktmp[:, n * 512:(n + 1) * 512],
                                 start=True, stop=True)
                (cp or cp_dve)(khat[:, n * 512:(n + 1) * 512], ps)

            def v_t(st, cp=None):
                # v rows of kv block (partitions 0:64) -> v_sb[st] [128, 64]
                ps = shp.tile([128, 64], bf16, tag="sh", name="psv")
                nc.tensor.transpose(
                    ps, kv_sb[0:64, st * 128:(st + 1) * 128], ID[0:64, 0:64])
                (cp or cp_dve)(v_sb[:, st, 0:HD], ps)

            def mask_diag(jt):
                lo = jt * 128
                nc.gpsimd.tensor_mul(expT[jt][:, lo:lo + 128],
                                     expT[jt][:, lo:lo + 128], TRI)

            def scores_seg(h, jt, c0, c1):
                # query columns [c0:c1) (512-aligned), one psum tile + exp
                lo = jt * 128
                s0 = max(lo, c0)
                if s0 >= c1:
                    return
                sc = scp.tile([128, 1024], f32, tag="sc", name="scr")
                for q0_ in range(c0, c1, 512):
                    qe = q0_ + 512
                    if qe <= lo:
                        continue
                    ss = max(q0_, lo)
                    nc.tensor.matmul(
                        sc[:, ss - c0:qe - c0], khat[:, lo:lo + 128],
                        qhat[h][:, ss:qe], start=True, stop=True)
                nc.scalar.activation(expT[jt][:, s0:c1], sc[:, s0 - c0:c1 - c0],
                                     EXP, scale=scale)
                if c0 <= lo:
                    mask_diag(jt)

            # incremental attnV: per (h, chunk) a persistent psum bank;
            # contributions added per key-tile right after its exp lands
            av_state = {}

            def attnv_open(h, c):
                av = avp.tile([128, 4 * 65], f32, tag="av", name=f"av{h}_{c}")
                av_state[(h, c)] = [av, None]

            def attnv_inc(h, c, jt):
                if jt >= 4 * c + 4:
                    return
                st = av_state[(h, c)]
                av = st[0]
                for it in range(max(4 * c, jt), 4 * c + 4):
                    sl = av[:, (it - 4 * c) * 65:(it - 4 * c) * 65 + 65]
                    mm = nc.tensor.matmul(
                        sl, expT[jt][:, it * 128:(it + 1) * 128],
                        v_sb[:, jt, :], start=(st[1] is None),
                        stop=(jt == it), skip_group_check=True)
                    if st[1] is None:
                        st[1] = mm
                    else:
                        add_dep_helper(mm.ins, st[1].ins, sync=False,
                                       reason="bank clear first")

            def attnv_norm(h, c):
                av = av_state.pop((h, c))[0]
                for it in range(4 * c, 4 * c + 4):
                    sl = av[:, (it - 4 * c) * 65:(it - 4 * c) * 65 + 65]
                    rc = rcp.tile([128, 1], f32, tag="rc", name="rc")
                    nc.vector.reciprocal(rc, sl[:, HD:HD + 1])
                    nc.vector.tensor_scalar_mul(
                        attn_n[it][:, h * HD:(h + 1) * HD], sl[:, 0:HD], rc)

            def transpose_quad(c, half):
                # 4 PE transposes of chunk c's attn tiles into one psum bank,
                # drained by a single 512-wide copy
                ps = shp.tile([128, 512], bf16, tag="sh", name="pst")
                for j in range(4):
                    it = 4 * c + j
                    nc.tensor.transpose(
                        ps[:, j * 128:(j + 1) * 128],
                        attn_n[it][:, half * 128:(half + 1) * 128], ID)
                nc.vector.tensor_copy(aT[half][:, c * 512:(c + 1) * 512], ps)

            def oproj_hc(c, hc, copy_eng):
                ps = shp.tile([128, 512], f32, tag="sh", name="pso")
                for kc2 in range(2):
                    nc.tensor.matmul(
                        ps, wo_sb[:, kc2, hc * 128:(hc + 1) * 128],
                        aT[kc2][:, c * 512:(c + 1) * 512],
                        start=(kc2 == 0), stop=(kc2 == 1))
                ot = otp.tile([128, 512], bf16, tag="ot", name="otst")
                copy_eng(ot, ps)
                nc.sync.dma_start(
                    oTd[hc * 128:(hc + 1) * 128, c * 512:(c + 1) * 512], ot)

            def oproj_pair(c, hc0, copy_eng):
                # tail variant: two hc blocks through one borrowed 2-bank
                # scores tile, drained by a single wide copy
                ps = scp.tile([128, 1024], f32, tag="sc", name="psow")
                for j in range(2):
                    hc = hc0 + j
                    for kc2 in range(2):
                        nc.tensor.matmul(
                            ps[:, j * 512:(j + 1) * 512],
                            wo_sb[:, kc2, hc * 128:(hc + 1) * 128],
                            aT[kc2][:, c * 512:(c + 1) * 512],
                            start=(kc2 == 0), stop=(kc2 == 1))
                ot = otp.tile([128, 1024], bf16, tag="otw", name="otw")
                copy_eng(ot, ps)
                for j in range(2):
                    hc = hc0 + j
                    nc.sync.dma_start(
                        oTd[hc * 128:(hc + 1) * 128, c * 512:(c + 1) * 512],
                        ot[:, j * 512:(j + 1) * 512])

            cp_dve = nc.vector.tensor_copy
            cp_act = nc.scalar.copy

            # ---- lead-in: chunk 0 of kv + q01; psum->sbuf copies on the
            # scalar engine, which is idle before the exp stream ----
            proj_block(0, 0, kv_sb, cp=cp_act)
            proj_block(1, 0, q01_sb, cp=cp_act)
            expand(EP1, kv_sb, ktmp, 0)
            expand(EP0, q01_sb, qhat[0], 0)
            fold(0, cp=cp_act)

            from collections import defaultdict
            fillA = defaultdict(list)
            fillB = defaultdict(list)

            # stripe A fillers (budget ~5.3us ACT per head window); chain-
            # dependent ops spaced >=2 jts apart, 8-matmul projections split
            # into halves (adjacent slots, no intervening shp allocation)
            # A0b: v transposes + q1 expands + q23 chunk 0
            for st in range(4, 6):
                fillA[(0, 1)].append(lambda st=st: v_t(st))
            for st in range(6, 8):
                fillA[(0, 2)].append(lambda st=st: v_t(st))
            fillA[(0, 4)].append(lambda: expand(EP1, q01_sb, qhat[1], 0))
            fillA[(0, 5)].append(lambda: expand(EP1, q01_sb, qhat[1], 1))
            fillA[(0, 6)].append(lambda: proj_block(2, 0, q23_sb, part=0))
            fillA[(0, 7)].append(lambda: proj_block(2, 0, q23_sb, part=1))
            # A1: q23 chunk 1 + q2/q3 expands (needed by A2/A3)
            fillA[(1, 0)].append(lambda: proj_block(2, 1, q23_sb, part=0))
            fillA[(1, 1)].append(lambda: proj_block(2, 1, q23_sb, part=1))
            fillA[(1, 2)].append(lambda: expand(EP0, q23_sb, qhat[2], 0))
            fillA[(1, 4)].append(lambda: expand(EP0, q23_sb, qhat[2], 1))
            fillA[(1, 5)].append(lambda: expand(EP1, q23_sb, qhat[3], 0))
            fillA[(1, 7)].append(lambda: expand(EP1, q23_sb, qhat[3], 1))
            # A2: q01 chunks 2,3 + first aT quads
            fillA[(2, 0)].append(lambda: proj_block(1, 2, q01_sb, part=0))
            fillA[(2, 1)].append(lambda: proj_block(1, 2, q01_sb, part=1))
            fillA[(2, 2)].append(lambda: transpose_quad(0, 0))
            fillA[(2, 3)].append(lambda: proj_block(1, 3, q01_sb, part=0))
            fillA[(2, 4)].append(lambda: proj_block(1, 3, q01_sb, part=1))
            fillA[(2, 5)].append(lambda: expand(EP0, q01_sb, qhat[0], 2))
            fillA[(2, 6)].append(lambda: transpose_quad(1, 0))
            fillA[(2, 7)].append(lambda: expand(EP1, q01_sb, qhat[1], 2))
            # A3: q0/q1 expands for stripe B
            fillA[(3, 0)].append(lambda: expand(EP0, q01_sb, qhat[0], 3))
            fillA[(3, 2)].append(lambda: expand(EP1, q01_sb, qhat[1], 3))
            fillA[(3, 5)].append(lambda: transpose_quad(0, 1))

            # B0: kv chunks 2,3 chains (khat tiles 8-15, v tiles 8-15 are
            # needed from B jt8/jt12) + last chunk-1 quad
            fillB[(0, 0)].append(lambda: transpose_quad(1, 1))
            fillB[(0, 0)].append(lambda: proj_block(0, 2, kv_sb, part=0))
            fillB[(0, 1)].append(lambda: proj_block(0, 2, kv_sb, part=1))
            fillB[(0, 2)].append(lambda: expand(EP1, kv_sb, ktmp, 2))
            fillB[(0, 4)].append(lambda: fold(2))
            for st in range(8, 10):
                fillB[(0, 4)].append(lambda st=st: v_t(st))
            for st in range(10, 12):
                fillB[(0, 5)].append(lambda st=st: v_t(st))
            fillB[(0, 5)].append(lambda: proj_block(0, 3, kv_sb, part=0))
            fillB[(0, 6)].append(lambda: proj_block(0, 3, kv_sb, part=1))
            fillB[(0, 7)].append(lambda: expand(EP1, kv_sb, ktmp, 3))
            fillB[(0, 9)].append(lambda: fold(3))
            for st in range(12, 16):
                fillB[(0, 10)].append(lambda st=st: v_t(st))
            # B1: oproj group 0 (1 hc/jt) + q23 chunks 2,3
            for i in range(KC):
                fillB[(1, i)].append(
                    lambda hc=i: oproj_hc(0, hc, cp_dve))
            fillB[(1, 9)].append(lambda: proj_block(2, 2, q23_sb))
            fillB[(1, 11)].append(lambda: expand(EP0, q23_sb, qhat[2], 2))
            fillB[(1, 13)].append(lambda: proj_block(2, 3, q23_sb))
            fillB[(1, 14)].append(lambda: transpose_quad(2, 0))
            fillB[(1, 15)].append(lambda: expand(EP0, q23_sb, qhat[2], 3))
            # B2: oproj group 1 + q3 expands (needed by B3)
            for i in range(KC):
                fillB[(2, i)].append(
                    lambda hc=i: oproj_hc(1, hc, cp_dve))
            fillB[(2, 9)].append(lambda: transpose_quad(3, 0))
            fillB[(2, 11)].append(lambda: expand(EP1, q23_sb, qhat[3], 2))
            fillB[(2, 13)].append(lambda: expand(EP1, q23_sb, qhat[3], 3))
            # B3: group 2 transposes + first oproj hcs
            fillB[(3, 12)].append(lambda: transpose_quad(2, 1))
            for i in range(2):
                fillB[(3, 14 + i)].append(
                    lambda hc=i: oproj_hc(2, hc, cp_dve))

            # ---- stripe A: query columns [0:1024] ----
            # fillers go BEFORE the scores matmuls: engine semaphore updates
            # are batched, so the exp must be the instruction right after its
            # producers or it waits on unrelated later matmuls.
            # h0 warms up on columns [0:512] (chunk-0 data only) while the
            # chunk-1 projection chain runs as its filler.
            attnv_open(0, 0)
            for st in range(4):
                v_t(st, cp=cp_act)
            for jt in range(4):
                if jt == 0:
                    proj_block(0, 1, kv_sb, part=0)
                if jt == 1:
                    proj_block(0, 1, kv_sb, part=1)
                    expand(EP1, kv_sb, ktmp, 1)
                if jt == 2:
                    proj_block(1, 1, q01_sb, part=0)
                if jt == 3:
                    proj_block(1, 1, q01_sb, part=1)
                scores_seg(0, jt, 0, 512)
                attnv_inc(0, 0, jt)
                if jt == 3:
                    attnv_norm(0, 0)
            expand(EP0, q01_sb, qhat[0], 1)
            fold(1)
            attnv_open(0, 1)
            for jt in range(8):
                for f in fillA[(0, jt)]:
                    f()
                scores_seg(0, jt, 512, 1024)
                attnv_inc(0, 1, jt)
                if jt == 7:
                    attnv_norm(0, 1)
            for h in range(1, G):
                attnv_open(h, 0)
                attnv_open(h, 1)
                for jt in range(8):
                    for f in fillA[(h, jt)]:
                        f()
                    scores_seg(h, jt, 0, 1024)
                    attnv_inc(h, 0, jt)
                    attnv_inc(h, 1, jt)
                    if jt == 3:
                        attnv_norm(h, 0)
                    if jt == 7:
                        attnv_norm(h, 1)

            # ---- stripe B: query columns [1024:2048] ----
            for h in range(G):
                attnv_open(h, 2)
                attnv_open(h, 3)
                for jt in range(NST):
                    for f in fillB[(h, jt)]:
                        f()
                    scores_seg(h, jt, 1024, 2048)
                    attnv_inc(h, 2, jt)
                    attnv_inc(h, 3, jt)
                    if jt == 11:
                        attnv_norm(h, 2)
                    if jt == 15:
                        attnv_norm(h, 3)

            # ---- tail: last attn chunk's transposes + remaining oproj ----
            transpose_quad(3, 1)
            for i, hc0 in enumerate((2, 4, 6)):
                oproj_pair(2, hc0, cp_act if i % 2 == 0 else cp_dve)
            for i, hc0 in enumerate((0, 2, 4, 6)):
                oproj_pair(3, hc0, cp_act if i % 2 == 1 else cp_dve)

    nc.finalize()
    return nc


def _host_inputs(hidden_states, position_ids, wq, wk, wv, wo):
    """Build the 8 per-core input maps."""
    # expansion matrices: out[m] = src[sel(m)] with sel via one-hot columns
    EP0 = np.zeros((128, 128), np.float32)
    EP1 = np.zeros((128, 128), np.float32)
    for m in range(64):
        EP0[m, m] = 1.0
        EP0[(m + 32) % 64, 64 + m] = 1.0
        EP1[64 + m, m] = 1.0
        EP1[64 + (m + 32) % 64, 64 + m] = 1.0

    dupJ = np.zeros((128, 128), np.float32)
    for p in range(128):
        dupJ[p, p % 64] = 1.0
        dupJ[p, p % 64 + 64] = 1.0
    ident = np.eye(128, dtype=np.float32)
    trimask = np.triu(np.ones((128, 128), np.float32))

    in_maps = []
    for core in range(N_CORES):
        b, kv = core // NKV, core % NKV
        xT = np.ascontiguousarray(hidden_states[b].T).astype(BF16)

        # packed blocks: [v; k], [q0; q1], [q2; q3]
        wvh = wv[kv * HD:(kv + 1) * HD]
        wkh = wk[kv * HD:(kv + 1) * HD]
        cols = [wvh.T, wkh.T]
        for i in range(G):
            h = kv * G + i
            cols.append(wq[h * HD:(h + 1) * HD].T)
        wqkvT = np.ascontiguousarray(np.concatenate(cols, axis=1)).astype(BF16)

        woT = np.ascontiguousarray(
            wo[:, kv * G * HD:(kv + 1) * G * HD].T).astype(BF16)

        inv = 1.0 / (THETA ** (np.arange(0, HD, 2, dtype=np.float32) / HD))
        freqs = position_ids[b].astype(np.float32)[:, None] * inv[None, :]
        emb = np.concatenate([freqs, freqs], axis=-1)       # [S, 64]
        cos_t = np.cos(emb).T                               # [64, S]
        ssin_t = np.sin(emb).T.copy()
        ssin_t[:32] *= -1.0                                 # sign of rotate_half
        cs = np.concatenate([cos_t, ssin_t], axis=0)        # [128, S]

        cst = np.concatenate(
            [EP0, EP1, dupJ, ident, trimask, cs], axis=1).astype(BF16)

        in_maps.append({
            "xT": xT, "wqkvT": wqkvT, "cst": np.ascontiguousarray(cst),
            "woT": woT,
        })
    return in_maps


_NC_CACHE = {}


def run_cores(in_maps, trace=False, trace_kwargs=None):
    from concourse.bass_utils import run_bass_kernel_spmd
    if "nc" not in _NC_CACHE:
        _NC_CACHE["nc"] = _build_nc()
    nc = _NC_CACHE["nc"]
    return run_bass_kernel_spmd(
        nc, in_maps, core_ids=list(range(N_CORES)),
        trace=trace, **(trace_kwargs or {}))


def kernel(hidden_states, attention_mask, position_ids, wq, wk, wv, wo):
    hidden_states = np.asarray(hidden_states, dtype=np.float32)
    position_ids = np.asarray(position_ids)
    wq = np.asarray(wq, dtype=np.float32)
    wk = np.asarray(wk, dtype=np.float32)
    wv = np.asarray(wv, dtype=np.float32)
    wo = np.asarray(wo, dtype=np.float32)

    in_maps = _host_inputs(hidden_states, position_ids, wq, wk, wv, wo)
    res = run_cores(in_maps)

    out = np.zeros((B, S, H), np.float32)
    for core in range(N_CORES):
        b = core // NKV
        out[b] += res.results[core]["oT"].T.astype(np.float32)
    return out


# revision 54
# speedup vs baseline: 1.1528x; 1.0077x over previous
"""Trainium2 Bass kernel for MimiAttention (GQA + RoPE + causal softmax).

Problem: B=2, S=2048, H=1024, NH=16 q-heads, NKV=4 kv-heads, HD=64.
Sharding: 8 cores = 2 (batch) x 4 (kv-group).  Each core computes one batch's
attention for one GQA group (4 q-heads sharing 1 kv head) and the partial
o-projection for those heads; the host sums the 4 partials per batch.

Per-core pipeline (all matmuls bf16 in / fp32 psum out):
  1. Packed QKV projection: one 128-out block holds [v; k], two blocks hold
     [q0; q1] and [q2; q3] (output width is free on the PE; only the moving
     dim costs cycles), so the whole projection is 3 blocks instead of the
     5 + separate-V of the naive RoPE-doubled layout.
  2. RoPE via PE "expand" matmuls: EP0/EP1 are constant [I | Perm] matrices
     producing [q_h; perm(q_h)] (128 rows) from a packed block; one DVE
     multiply with cs = [cos; +-sin] then yields qhat = [q cos; rot(q) sin].
     The scores contraction uses the 128-dim identity
       qhat . dup(k_rot) = q_rot . k_rot
     with khat = J @ ktmp (fold+duplicate in one matmul).
  3. Scores computed TRANSPOSED (scoresT[j,i]) into wide 2-bank PSUM tiles;
     exp on ACT in up-to-1024-wide instructions (the per-instruction PSUM /
     SBUF access overhead is ~185 ns, so wide exps matter).  Head 0 walks
     the triangle column-major (per 512 query panel) so the exp stream
     starts as soon as the first xt pieces land; heads 1-3 go row-major.
     Causal masking: one in-place Pool multiply of the diagonal tile.
  4. attnV accumulates av[i, d | denom] per 4-tile query chunk in a single
     PSUM bank ([128, 4*65] f32); reciprocal + per-partition scalar multiply
     normalizes into attn_n.
  5. attn transposes for the o-projection go through the DMA xbar
     (dma_start_transpose, no PE/PSUM involved); o-projection psum drains
     on DVE/ACT; output written transposed ([h, s]); host transposes back
     and sums partials.
"""

import numpy as np
import ml_dtypes

B, S, H = 2, 2048, 1024
NH, NKV, HD = 16, 4, 64
G = NH // NKV            # 4 q-heads per kv head
THETA = 10000.0
N_CORES = 8

BF16 = ml_dtypes.bfloat16


def _build_nc():
    import contextlib
    import concourse.mybir as mybir
    import concourse.tile as tile
    from concourse.tile import add_dep_helper
    from concourse import bacc

    f32 = mybir.dt.float32
    bf16 = mybir.dt.bfloat16
    EXP = mybir.ActivationFunctionType.Exp

    nc = bacc.Bacc("TRN2", target_bir_lowering=False)

    xTd = nc.dram_tensor("xT", [H, S], bf16, kind="ExternalInput")
    wqkvd = nc.dram_tensor("wqkvT", [H, 384], bf16, kind="ExternalInput")
    cstd = nc.dram_tensor("cst", [128, 2688], bf16, kind="ExternalInput")
    wod = nc.dram_tensor("woT", [G * HD, H], bf16, kind="ExternalInput")
    oTd = nc.dram_tensor("oT", [H, S], bf16, kind="ExternalOutput")

    KC = H // 128         # 8 contraction chunks
    NST = S // 128        # 16 tiles of 128
    scale = float(1.0 / np.sqrt(HD))

    with tile.TileContext(nc) as tc:
        ctx = contextlib.ExitStack()
        with ctx:
            consts = ctx.enter_context(tc.tile_pool(name="consts", bufs=1))
            acts = ctx.enter_context(tc.tile_pool(name="acts", bufs=1))
            rcp = ctx.enter_context(tc.tile_pool(name="rcp", bufs=6))
            otp = ctx.enter_context(tc.tile_pool(name="ot", bufs=8))
            # PSUM: sc 2x[128,1024] (4 banks) + av 2x[128,260] (2 banks)
            # + sh 2x[128,512] (2 banks) = 8 banks.
            scp = ctx.enter_context(
                tc.tile_pool(name="ps_sc", bufs=2, space="PSUM"))
            avp = ctx.enter_context(
                tc.tile_pool(name="ps_av", bufs=2, space="PSUM"))
            shp = ctx.enter_context(
                tc.tile_pool(name="ps_sh", bufs=2, space="PSUM"))

            # ---- input DMAs, ordered for earliest first matmul ----
            wqr = wqkvd.rearrange("(kc p) m -> p kc m", p=128)
            xtr = xTd.rearrange("(kc p) m -> p kc m", p=128)
            wqkv_sb = consts.tile([128, KC, 384], bf16, tag="wqkv")
            cst_sb = consts.tile([128, 2688], bf16, tag="cst")
            xt_sb = consts.tile([128, KC, S], bf16, tag="xt")
            wo_sb = consts.tile([128, 2, H], bf16, tag="wo")

            def xt_dma(i):
                nc.sync.dma_start(xt_sb[:, :, i * 512:(i + 1) * 512],
                                  xtr[:, :, i * 512:(i + 1) * 512])

            nc.sync.dma_start(cst_sb[:, 0:384], cstd[:, 0:384])
            nc.sync.dma_start(wqkv_sb[:, :, 0:256], wqr[:, :, 0:256])
            xt_dma(0)
            xt_dma(1)
            nc.sync.dma_start(cst_sb[:, 384:2688], cstd[:, 384:2688])
            xt_dma(2)
            xt_dma(3)
            nc.sync.dma_start(wqkv_sb[:, :, 256:384], wqr[:, :, 256:384])
            nc.sync.dma_start(wo_sb, wod.rearrange("(kc p) m -> p kc m", p=128))

            EP0 = cst_sb[:, 0:128]
            EP1 = cst_sb[:, 128:256]
            DJ = cst_sb[:, 256:384]
            ID = cst_sb[:, 384:512]
            TRI = cst_sb[:, 512:640]
            CS = cst_sb[:, 640:2688]

            # ---- SBUF activations ----
            kv_sb = acts.tile([128, S], bf16, tag="kv")
            q01_sb = acts.tile([128, S], bf16, tag="q01")
            q23_sb = acts.tile([128, S], bf16, tag="q23")
            qhat = [acts.tile([128, S], bf16, tag=f"qh{m}", name=f"qhat{m}")
                    for m in range(G)]
            khat = acts.tile([128, S], bf16, tag="khat")
            ktmp = acts.tile([128, S], bf16, tag="ktmp")
            v_sb = acts.tile([128, NST, HD + 1], bf16, tag="vsb")
            expT = [acts.tile([128, S], bf16, tag=f"e{jt}", name=f"expT{jt}")
                    for jt in range(NST)]
            attn_n = [acts.tile([128, G * HD], bf16, tag=f"an{it}",
                                name=f"attn{it}")
                      for it in range(NST)]
            aT = [acts.tile([128, S], bf16, tag=f"aT{c}", name=f"aTc{c}")
                  for c in range(2)]
            wup = acts.tile([128, 512], bf16, tag="wup")
            nc.vector.memset(wup, 0.0)
            nc.vector.memset(v_sb[:, :, HD:HD + 1], 1.0)

            cp_dve = nc.vector.tensor_copy
            cp_act = nc.scalar.copy

            # PE p-state warmup: junk matmuls bridge the DMA wait so the
            # tensor engine is at full clock when real projections start.
            for _ in range(16):
                pw = scp.tile([128, 1024], f32, tag="sc", name="pwarm")
                nc.tensor.matmul(pw[:, 0:512], wup[:, 0:128], wup,
                                 start=True, stop=True)

            # ---- helpers ----
            _proj_ps = {}

            def proj_block(blk, n, dst, cp=None, part=2):
                # part 0: first half of the contraction; 1: second half +
                # drain; 2: whole block.  Halves let an 8-matmul projection
                # spread over two filler slots.
                if part in (0, 2):
                    _proj_ps[(blk, n)] = shp.tile(
                        [128, 512], f32, tag="sh", name="psb")
                ps = _proj_ps[(blk, n)]
                kcs = {0: range(0, 4), 1: range(4, KC), 2: range(KC)}[part]
                for kc in kcs:
                    nc.tensor.matmul(
                        ps, wqkv_sb[:, kc, blk * 128:(blk + 1) * 128],
                        xt_sb[:, kc, n * 512:(n + 1) * 512],
                        start=(kc == 0), stop=(kc == KC - 1))
                if part in (1, 2):
                    (cp or cp_dve)(dst[:, n * 512:(n + 1) * 512], ps)
                    del _proj_ps[(blk, n)]

            def expand(ep, src, dst, n):
                # dst chunk = (ep.T @ src chunk) * CS  -> [x; perm x] * cs
                ps = shp.tile([128, 512], f32, tag="sh", name="pse")
                nc.tensor.matmul(ps, ep, src[:, n * 512:(n + 1) * 512],
                                 start=True, stop=True)
                nc.vector.tensor_mul(dst[:, n * 512:(n + 1) * 512], ps,
                                     CS[:, n * 512:(n + 1) * 512])

            def fold(n, cp=None):
                ps = shp.tile([128, 512], f32, tag="sh", name="psf")
                nc.tensor.matmul(ps, DJ, ktmp[:, n * 512:(n + 1) * 512],
                                 start=True, stop=True)
                (cp or cp_dve)(khat[:, n * 512:(n + 1) * 512], ps)

            def v_t(st, cp=None):
                # v rows of kv block (partitions 0:64) -> v_sb[st] [128, 64]
                ps = shp.tile([128, 64], bf16, tag="sh", name="psv")
                nc.tensor.transpose(
                    ps, kv_sb[0:64, st * 128:(st + 1) * 128], ID[0:64, 0:64])
                (cp or cp_dve)(v_sb[:, st, 0:HD], ps)

            def mask_diag(jt):
                lo = jt * 128
                nc.gpsimd.tensor_mul(expT[jt][:, lo:lo + 128],
                                     expT[jt][:, lo:lo + 128], TRI)

            def scores_seg(h, jt, c0, c1):
                # query columns [c0:c1) (512-aligned), one psum tile + exp
                lo = jt * 128
                s0 = max(lo, c0)
                if s0 >= c1:
                    return
                sc = scp.tile([128, 1024], f32, tag="sc", name="scr")
                for q0_ in range(c0, c1, 512):
                    qe = q0_ + 512
                    if qe <= lo:
                        continue
                    ss = max(q0_, lo)
                    nc.tensor.matmul(
                        sc[:, ss - c0:qe - c0], khat[:, lo:lo + 128],
                        qhat[h][:, ss:qe], start=True, stop=True)
                nc.scalar.activation(expT[jt][:, s0:c1], sc[:, s0 - c0:c1 - c0],
                                     EXP, scale=scale)
                if c0 <= lo:
                    mask_diag(jt)

            # incremental attnV: per (h, chunk) a persistent psum bank;
            # contributions added per key-tile right after its exp lands
            av_state = {}

            def attnv_open(h, c):
                av = avp.tile([128, 4 * 65], f32, tag="av", name=f"av{h}_{c}")
                av_state[(h, c)] = [av, None]

            def attnv_inc(h, c, jt):
                if jt >= 4 * c + 4:
                    return
                st = av_state[(h, c)]
                av = st[0]
                for it in range(max(4 * c, jt), 4 * c + 4):
                    sl = av[:, (it - 4 * c) * 65:(it - 4 * c) * 65 + 65]
                    mm = nc.tensor.matmul(
                        sl, expT[jt][:, it * 128:(it + 1) * 128],
                        v_sb[:, jt, :], start=(st[1] is None),
                        stop=(jt == it), skip_group_check=True)
                    if st[1] is None:
                        st[1] = mm
                    else:
                        add_dep_helper(mm.ins, st[1].ins, sync=False,
                                       reason="bank clear first")

            def attnv_norm(h, c):
                av = av_state.pop((h, c))[0]
                for it in range(4 * c, 4 * c + 4):
                    sl = av[:, (it - 4 * c) * 65:(it - 4 * c) * 65 + 65]
                    rc = rcp.tile([128, 1], f32, tag="rc", name="rc")
                    nc.vector.reciprocal(rc, sl[:, HD:HD + 1])
                    nc.vector.tensor_scalar_mul(
                        attn_n[it][:, h * HD:(h + 1) * HD], sl[:, 0:HD], rc)

            def transpose_quad(c, half):
                # 4 PE transposes of chunk c's attn tiles into one psum bank,
                # drained by a single 512-wide copy
                ps = shp.tile([128, 512], bf16, tag="sh", name="pst")
                for j in range(4):
                    it = 4 * c + j
                    nc.tensor.transpose(
                        ps[:, j * 128:(j + 1) * 128],
                        attn_n[it][:, half * 128:(half + 1) * 128], ID)
                nc.vector.tensor_copy(aT[half][:, c * 512:(c + 1) * 512], ps)

            def oproj_hc(c, hc, copy_eng):
                ps = shp.tile([128, 512], f32, tag="sh", name="pso")
                for kc2 in range(2):
                    nc.tensor.matmul(
                        ps, wo_sb[:, kc2, hc * 128:(hc + 1) * 128],
                        aT[kc2][:, c * 512:(c + 1) * 512],
                        start=(kc2 == 0), stop=(kc2 == 1))
                ot = otp.tile([128, 512], bf16, tag="ot", name="otst")
                copy_eng(ot, ps)
                nc.sync.dma_start(
                    oTd[hc * 128:(hc + 1) * 128, c * 512:(c + 1) * 512], ot)

            def oproj_pair(c, hc0, copy_eng):
                # tail variant: two hc blocks through one borrowed 2-bank
                # scores tile, drained by a single wide copy
                ps = scp.tile([128, 1024], f32, tag="sc", name="psow")
                for j in range(2):
                    hc = hc0 + j
                    for kc2 in range(2):
                        nc.tensor.matmul(
                            ps[:, j * 512:(j + 1) * 512],
                            wo_sb[:, kc2, hc * 128:(hc + 1) * 128],
                            aT[kc2][:, c * 512:(c + 1) * 512],
                            start=(kc2 == 0), stop=(kc2 == 1))
                ot = otp.tile([128, 1024], bf16, tag="otw", name="otw")
                copy_eng(ot, ps)
                for j in range(2):
                    hc = hc0 + j
                    nc.sync.dma_start(
                        oTd[hc * 128:(hc + 1) * 128, c * 512:(c + 1) * 512],
                        ot[:, j * 512:(j + 1) * 512])

            cp_dve = nc.vector.tensor_copy
            cp_act = nc.scalar.copy

            # ---- lead-in: chunk 0 of kv + q01; psum->sbuf copies on the
            # scalar engine, which is idle before the exp stream ----
            proj_block(0, 0, kv_sb, cp=cp_act)
            proj_block(1, 0, q01_sb, cp=cp_act)
            expand(EP1, kv_sb, ktmp, 0)
            expand(EP0, q01_sb, qhat[0], 0)
            fold(0, cp=cp_act)

            from collections import defaultdict
            fillA = defaultdict(list)
            fillB = defaultdict(list)

            # stripe A fillers (budget ~5.3us ACT per head window); chain-
            # dependent ops spaced >=2 jts apart, 8-matmul projections split
            # into halves (adjacent slots, no intervening shp allocation)
            # A0b: v transposes + q1 expands + q23 chunk 0
            for st in range(4, 6):
                fillA[(0, 1)].append(lambda st=st: v_t(st))
            for st in range(6, 8):
                fillA[(0, 2)].append(lambda st=st: v_t(st))
            fillA[(0, 4)].append(lambda: expand(EP1, q01_sb, qhat[1], 0))
            fillA[(0, 5)].append(lambda: expand(EP1, q01_sb, qhat[1], 1))
            fillA[(0, 6)].append(lambda: proj_block(2, 0, q23_sb, part=0))
            fillA[(0, 7)].append(
                lambda: proj_block(2, 0, q23_sb, cp=cp_act, part=1))
            # A1: q23 chunk 1 + q2/q3 expands (needed by A2/A3)
            fillA[(1, 0)].append(lambda: proj_block(2, 1, q23_sb, part=0))
            fillA[(1, 1)].append(
                lambda: proj_block(2, 1, q23_sb, cp=cp_act, part=1))
            fillA[(1, 2)].append(lambda: expand(EP0, q23_sb, qhat[2], 0))
            fillA[(1, 4)].append(lambda: expand(EP0, q23_sb, qhat[2], 1))
            fillA[(1, 5)].append(lambda: expand(EP1, q23_sb, qhat[3], 0))
            fillA[(1, 7)].append(lambda: expand(EP1, q23_sb, qhat[3], 1))
            # A2: q01 chunks 2,3 + first aT quads
            fillA[(2, 0)].append(lambda: proj_block(1, 2, q01_sb, part=0))
            fillA[(2, 1)].append(
                lambda: proj_block(1, 2, q01_sb, cp=cp_act, part=1))
            fillA[(2, 2)].append(lambda: transpose_quad(0, 0))
            fillA[(2, 3)].append(lambda: proj_block(1, 3, q01_sb, part=0))
            fillA[(2, 4)].append(
                lambda: proj_block(1, 3, q01_sb, cp=cp_act, part=1))
            fillA[(2, 5)].append(lambda: expand(EP0, q01_sb, qhat[0], 2))
            fillA[(2, 6)].append(lambda: transpose_quad(1, 0))
            fillA[(2, 7)].append(lambda: expand(EP1, q01_sb, qhat[1], 2))
            # A3: q0/q1 expands for stripe B
            fillA[(3, 0)].append(lambda: expand(EP0, q01_sb, qhat[0], 3))
            fillA[(3, 2)].append(lambda: expand(EP1, q01_sb, qhat[1], 3))
            fillA[(3, 5)].append(lambda: transpose_quad(0, 1))

            # B0: kv chunks 2,3 chains (khat tiles 8-15, v tiles 8-15 are
            # needed from B jt8/jt12) + last chunk-1 quad
            fillB[(0, 0)].append(lambda: transpose_quad(1, 1))
            fillB[(0, 0)].append(lambda: proj_block(0, 2, kv_sb, part=0))
            fillB[(0, 1)].append(lambda: proj_block(0, 2, kv_sb, part=1))
            fillB[(0, 2)].append(lambda: expand(EP1, kv_sb, ktmp, 2))
            fillB[(0, 4)].append(lambda: fold(2))
            for st in range(8, 10):
                fillB[(0, 4)].append(lambda st=st: v_t(st))
            for st in range(10, 12):
                fillB[(0, 5)].append(lambda st=st: v_t(st))
            fillB[(0, 5)].append(lambda: proj_block(0, 3, kv_sb, part=0))
            fillB[(0, 6)].append(lambda: proj_block(0, 3, kv_sb, part=1))
            fillB[(0, 7)].append(lambda: expand(EP1, kv_sb, ktmp, 3))
            fillB[(0, 9)].append(lambda: fold(3))
            for st in range(12, 16):
                fillB[(0, 10)].append(lambda st=st: v_t(st))
            # B1: oproj group 0 (1 hc/jt) + q23 chunks 2,3
            for i in range(KC):
                fillB[(1, i)].append(
                    lambda hc=i: oproj_hc(0, hc, cp_dve))
            fillB[(1, 9)].append(lambda: proj_block(2, 2, q23_sb))
            fillB[(1, 11)].append(lambda: expand(EP0, q23_sb, qhat[2], 2))
            fillB[(1, 13)].append(lambda: proj_block(2, 3, q23_sb))
            fillB[(1, 14)].append(lambda: transpose_quad(2, 0))
            fillB[(1, 15)].append(lambda: expand(EP0, q23_sb, qhat[2], 3))
            # B2: oproj group 1 + q3 expands (needed by B3)
            for i in range(KC):
                fillB[(2, i)].append(
                    lambda hc=i: oproj_hc(1, hc, cp_dve))
            fillB[(2, 9)].append(lambda: transpose_quad(3, 0))
            fillB[(2, 11)].append(lambda: expand(EP1, q23_sb, qhat[3], 2))
            fillB[(2, 13)].append(lambda: expand(EP1, q23_sb, qhat[3], 3))
            # B3: group 2 transposes + first oproj hcs
            fillB[(3, 13)].append(lambda: transpose_quad(2, 1))
            for i in range(2):
                fillB[(3, 14 + i)].append(
                    lambda hc=i: oproj_hc(2, hc, cp_dve))

            # ---- stripe A: query columns [0:1024] ----
            # fillers go BEFORE the scores matmuls: engine semaphore updates
            # are batched, so the exp must be the instruction right after its
            # producers or it waits on unrelated later matmuls.
            # h0 warms up on columns [0:512] (chunk-0 data only) while the
            # chunk-1 projection chain runs as its filler.
            attnv_open(0, 0)
            for st in range(4):
                v_t(st, cp=cp_act)
            for jt in range(4):
                if jt == 0:
                    proj_block(0, 1, kv_sb, part=0)
                if jt == 1:
                    proj_block(0, 1, kv_sb, part=1)
                    expand(EP1, kv_sb, ktmp, 1)
                if jt == 2:
                    proj_block(1, 1, q01_sb, part=0)
                if jt == 3:
                    proj_block(1, 1, q01_sb, part=1)
                if jt >= 1:
                    attnv_inc(0, 0, jt - 1)
                scores_seg(0, jt, 0, 512)
            attnv_inc(0, 0, 3)
            attnv_norm(0, 0)
            expand(EP0, q01_sb, qhat[0], 1)
            fold(1)
            attnv_open(0, 1)
            for jt in range(8):
                for f in fillA[(0, jt)]:
                    f()
                if jt >= 1:
                    attnv_inc(0, 1, jt - 1)
                scores_seg(0, jt, 512, 1024)
            attnv_inc(0, 1, 7)
            attnv_norm(0, 1)
            for h in range(1, G):
                attnv_open(h, 0)
                attnv_open(h, 1)
                for jt in range(8):
                    for f in fillA[(h, jt)]:
                        f()
                    if jt >= 1:
                        attnv_inc(h, 0, jt - 1)
                        attnv_inc(h, 1, jt - 1)
                    if jt == 4:
                        attnv_norm(h, 0)
                    scores_seg(h, jt, 0, 1024)
                attnv_inc(h, 1, 7)
                attnv_norm(h, 1)

            # ---- stripe B: query columns [1024:2048] ----
            for h in range(G):
                attnv_open(h, 2)
                attnv_open(h, 3)
                for jt in range(NST):
                    for f in fillB[(h, jt)]:
                        f()
                    if jt >= 1:
                        attnv_inc(h, 2, jt - 1)
                        attnv_inc(h, 3, jt - 1)
                    if jt == 12:
                        attnv_norm(h, 2)
                    scores_seg(h, jt, 1024, 2048)
                attnv_inc(h, 3, 15)
                attnv_norm(h, 3)

            # ---- tail: last attn chunk's transposes + remaining oproj.
            # group-2 pairs first: they are ready before norm(3,3) lands ----
            for i, hc0 in enumerate((2, 4, 6)):
                oproj_pair(2, hc0, cp_act if i % 2 == 0 else cp_dve)
            transpose_quad(3, 1)
            for i, hc0 in enumerate((0, 2, 4, 6)):
                oproj_pair(3, hc0, cp_act if i % 2 == 1 else cp_dve)

    nc.finalize()
    return nc


def _host_inputs(hidden_states, position_ids, wq, wk, wv, wo):
    """Build the 8 per-core input maps."""
    # expansion matrices: out[m] = src[sel(m)] with sel via one-hot columns
    EP0 = np.zeros((128, 128), np.float32)
    EP1 = np.zeros((128, 128), np.float32)
    for m in range(64):
        EP0[m, m] = 1.0
        EP0[(m + 32) % 64, 64 + m] = 1.0
        EP1[64 + m, m] = 1.0
        EP1[64 + (m + 32) % 64, 64 + m] = 1.0

    dupJ = np.zeros((128, 128), np.float32)
    for p in range(128):
        dupJ[p, p % 64] = 1.0
        dupJ[p, p % 64 + 64] = 1.0
    ident = np.eye(128, dtype=np.float32)
    trimask = np.triu(np.ones((128, 128), np.float32))

    in_maps = []
    for core in range(N_CORES):
        b, kv = core // NKV, core % NKV
        xT = np.ascontiguousarray(hidden_states[b].T).astype(BF16)

        # packed blocks: [v; k], [q0; q1], [q2; q3]
        wvh = wv[kv * HD:(kv + 1) * HD]
        wkh = wk[kv * HD:(kv + 1) * HD]
        cols = [wvh.T, wkh.T]
        for i in range(G):
            h = kv * G + i
            cols.append(wq[h * HD:(h + 1) * HD].T)
        wqkvT = np.ascontiguousarray(np.concatenate(cols, axis=1)).astype(BF16)

        woT = np.ascontiguousarray(
            wo[:, kv * G * HD:(kv + 1) * G * HD].T).astype(BF16)

        inv = 1.0 / (THETA ** (np.arange(0, HD, 2, dtype=np.float32) / HD))
        freqs = position_ids[b].astype(np.float32)[:, None] * inv[None, :]
        emb = np.concatenate([freqs, freqs], axis=-1)       # [S, 64]
        cos_t = np.cos(emb).T                               # [64, S]
        ssin_t = np.sin(emb).T.copy()
        ssin_t[:32] *= -1.0                                 # sign of rotate_half
        cs = np.concatenate([cos_t, ssin_t], axis=0)        # [128, S]

        cst = np.concatenate(
            [EP0, EP1, dupJ, ident, trimask, cs], axis=1).astype(BF16)

        in_maps.append({
            "xT": xT, "wqkvT": wqkvT, "cst": np.ascontiguousarray(cst),
            "woT": woT,
        })
    return in_maps


_NC_CACHE = {}


def run_cores(in_maps, trace=False, trace_kwargs=None):
    from concourse.bass_utils import run_bass_kernel_spmd
    if "nc" not in _NC_CACHE:
        _NC_CACHE["nc"] = _build_nc()
    nc = _NC_CACHE["nc"]
    return run_bass_kernel_spmd(
        nc, in_maps, core_ids=list(range(N_CORES)),
        trace=trace, **(trace_kwargs or {}))


def kernel(hidden_states, attention_mask, position_ids, wq, wk, wv, wo):
    hidden_states = np.asarray(hidden_states, dtype=np.float32)
    position_ids = np.asarray(position_ids)
    wq = np.asarray(wq, dtype=np.float32)
    wk = np.asarray(wk, dtype=np.float32)
    wv = np.asarray(wv, dtype=np.float32)
    wo = np.asarray(wo, dtype=np.float32)

    in_maps = _host_inputs(hidden_states, position_ids, wq, wk, wv, wo)
    res = run_cores(in_maps)

    out = np.zeros((B, S, H), np.float32)
    for core in range(N_CORES):
        b = core // NKV
        out[b] += res.results[core]["oT"].T.astype(np.float32)
    return out
